# revision 1
# baseline (speedup 1.0000x reference)
"""RWKV GPT block kernel for 8 Trainium2 NeuronCores — fp8 DoubleRow v3.

Same quantization plan as v2 (r/cr/key/val plain fp8 DoubleRow; v/o 3-term
hi/lo fp8; LN stats via fp8 ones-matmuls), plus scheduling fixes:

- Batched DMAs: x loaded with one DMA per chunk per dtype ([128, KT, T]
  DRAM layout); per-oi weight panels combined into single tensors
  (Wvr = v-hi|v-lo|r, Woc = o-hi|o-lo, Wvc = val|cr) so each oi costs one
  descriptor; x1f spilled with one transposed-AP DMA per chunk.
- LN1 applies interleaved between per-chunk matmul sections so the DVE
  apply work of chunk i+1 hides under chunk i's PE chains.
- vlo/olo residual subs on GpSimd to unload the DVE.
- FFN: key matmuls grouped (8 uis per weight-hold) with the half-0 pass
  first so the group-0 half-0 chains (which only need LN2 chunks 0/1)
  cover the chunk-2 LN2 applies.
"""
import sys

sys.path.insert(0, "/opt/trn_rl_repo")
sys.path.insert(0, "/opt/pypackages")

import numpy as np
import ml_dtypes

H = 2048
KT = H // 128
OT = H // 128
UPT = 4 * H // 128
B = 4
T = 2048
TCORE = 1026
EPS = 1e-5
INV_H = 1.0 / H
SW = 32.0
SWV = 64.0
ISW = 1.0 / SW
ISWV = 1.0 / SWV
E4 = ml_dtypes.float8_e4m3

CH = [(1, 342), (342, 684), (684, 1026)]
ST1 = [(0, 342), (342, 684), (684, 1026)]
KG = 8                 # FFN key-group size (uis per weight hold)

_BUILD_CACHE = {}


def _panels(WT):
    IN, OUT = WT.shape
    kt, ot = IN // 128, OUT // 128
    return np.ascontiguousarray(
        WT.reshape(kt, 128, ot, 128).transpose(2, 1, 0, 3))


def _p8(W, scale):
    a = _panels(np.asarray(W, np.float32).T) * scale
    return a.astype(E4)


def _p8_hilo(W, scale):
    a = _panels(np.asarray(W, np.float32).T) * scale
    hi = a.astype(E4)
    lo = (a - hi.astype(np.float32)).astype(E4)
    return hi, lo


def _mix128(v):
    return np.ascontiguousarray(
        np.asarray(v, dtype=np.float32).reshape(-1)[:H].reshape(KT, 128).T)


def build():
    import contextlib

    import concourse.bacc as bacc
    import concourse.mybir as mybir
    import concourse.tile as tile

    F16 = mybir.dt.float16
    F32 = mybir.dt.float32
    F8 = mybir.dt.float8e4
    AF = mybir.ActivationFunctionType
    OP = mybir.AluOpType
    DR = mybir.MatmulPerfMode.DoubleRow

    nc = bacc.Bacc("TRN2", target_bir_lowering=False)

    xT = nc.dram_tensor("xT", [128, KT, TCORE], F16, kind="ExternalInput")
    xT8 = nc.dram_tensor("xT8", [3, 128, KT, 342], F8, kind="ExternalInput")
    Wvr = nc.dram_tensor("Wvr", [OT, 128, 3, KT, 128], F8, kind="ExternalInput")
    Woc = nc.dram_tensor("Woc", [OT, 128, 2, KT, 128], F8, kind="ExternalInput")
    Wkey8 = nc.dram_tensor("Wkey8", [UPT, 128, KT, 128], F8, kind="ExternalInput")
    Wvc = nc.dram_tensor("Wvc", [OT, 128, UPT + KT, 128], F8,
                         kind="ExternalInput")
    mixa = nc.dram_tensor("mixa", [128, 3, KT], F32, kind="ExternalInput")
    out = nc.dram_tensor("out", [OT, 128, 1024], F32, kind="ExternalOutput")
    x1f = nc.dram_tensor("x1f", [OT, 128, TCORE], F16, kind="Internal")

    with tile.TileContext(nc) as tc, contextlib.ExitStack() as g:
        cpool = g.enter_context(tc.tile_pool(name="consts", bufs=1))
        psg = contextlib.ExitStack()
        st = psg.enter_context(tc.tile_pool(name="st", bufs=1, space="PSUM"))
        mm = psg.enter_context(tc.tile_pool(name="mm", bufs=6, space="PSUM"))
        rows = g.enter_context(tc.tile_pool(name="rows", bufs=2))
        rsc = g.enter_context(tc.tile_pool(name="rsc", bufs=1))
        bcs = g.enter_context(tc.tile_pool(name="bcs", bufs=2))
        sqp = g.enter_context(tc.tile_pool(name="sqp", bufs=1))

        ones_r = cpool.tile([1, 128], F16)
        nc.vector.memset(ones_r[:], 1.0)
        ones8 = cpool.tile([128, 2, 128], F8)
        nc.vector.memset(ones8[:], 1.0)
        mixt = cpool.tile([128, 3, KT], F32)
        nc.sync.dma_start(mixt[:], mixa[:])
        mv = mixt[:, 0]
        mr = mixt[:, 1]
        mk = mixt[:, 2]

        def stats_rows(src8, n):
            s1 = st.tile([128, 512], F32, tag="s1")
            s2 = st.tile([128, 512], F32, tag="s2")
            sq = sqp.tile([128, KT, 342], F8, tag="sq")
            for ki in range(KT):
                nc.scalar.square(sq[:, ki, :n], src8[:, ki, :])
            for ki in range(0, KT, 2):
                nc.tensor.matmul(s1[:, :n], ones8[:], src8[:, ki : ki + 2, :],
                                 start=(ki == 0), stop=(ki == KT - 2),
                                 perf_mode=DR)
            for ki in range(0, KT, 2):
                nc.tensor.matmul(s2[:, :n], ones8[:], sq[:, ki : ki + 2, :n],
                                 start=(ki == 0), stop=(ki == KT - 2),
                                 perf_mode=DR)
            m = rsc.tile([1, 512], F32, tag="m")
            nc.vector.tensor_scalar_mul(m[:, :n], s1[0:1, :n], INV_H)
            var = rsc.tile([1, 512], F32, tag="var")
            nc.vector.tensor_scalar_mul(var[:, :n], s2[0:1, :n], INV_H)
            msq = rsc.tile([1, 512], F32, tag="msd")
            nc.vector.tensor_mul(msq[:, :n], m[:, :n], m[:, :n])
            nc.vector.tensor_sub(var[:, :n], var[:, :n], msq[:, :n])
            nc.vector.tensor_scalar_add(var[:, :n], var[:, :n], EPS)
            sd = rsc.tile([1, 512], F32, tag="msd")
            nc.scalar.sqrt(sd[:, :n], var[:, :n])
            a_rf = rsc.tile([1, 512], F32, tag="var")
            nc.vector.reciprocal(a_rf[:, :n], sd[:, :n])
            a_rc = rows.tile([1, 512], F16, tag="arow")
            nc.vector.tensor_copy(a_rc[:, :n], a_rf[:, :n])
            c_rc = rows.tile([1, 512], F16, tag="crow")
            nc.vector.scalar_tensor_tensor(
                c_rc[:, :n], m[:, :n], -1.0, a_rf[:, :n],
                op0=OP.mult, op1=OP.mult)
            return a_rc, c_rc

        def bcast2(a_rc, c_rc, n, prev, hn):
            off = hn - n
            abp = st.tile([128, 512], F32, tag="s1")
            cbp = st.tile([128, 512], F32, tag="s2")
            if off:
                pa_rc, pc_rc, pn = prev
                nc.tensor.matmul(abp[:, 0:1], ones_r[:], pa_rc[:, pn - 1 : pn],
                                 start=True, stop=True, skip_group_check=True)
                nc.tensor.matmul(cbp[:, 0:1], ones_r[:], pc_rc[:, pn - 1 : pn],
                                 start=True, stop=True, skip_group_check=True)
            nc.tensor.matmul(abp[:, off : off + n], ones_r[:], a_rc[:, :n],
                             start=True, stop=True, skip_group_check=True)
            nc.tensor.matmul(cbp[:, off : off + n], ones_r[:], c_rc[:, :n],
                             start=True, stop=True, skip_group_check=True)
            ab = bcs.tile([128, 512], F16, tag="ab")
            nc.scalar.copy(ab[:, :hn], abp[:, :hn])
            cb = bcs.tile([128, 512], F16, tag="cb")
            nc.scalar.copy(cb[:, :hn], cbp[:, :hn])
            return ab, cb

        with tc.tile_pool(name="cmp", bufs=1) as cmp_:
            cmt = cmp_.tile([128, KT, 1024], F8)

            with tc.tile_pool(name="ohip", bufs=1) as ohip, \
                 tc.tile_pool(name="olop", bufs=1) as olop, \
                 tc.tile_pool(name="xtlp", bufs=1) as xtlp:
                ohi = [ohip.tile([128, KT, pb - pa], F8, tag=f"ohi{ci}",
                                 name=f"ohi{ci}")
                       for ci, (pa, pb) in enumerate(CH)]
                olo = [olop.tile([128, KT, pb - pa], F8, tag=f"olo{ci}",
                                 name=f"olo{ci}")
                       for ci, (pa, pb) in enumerate(CH)]
                xtl = []
                for ci, (sa, sb) in enumerate(ST1):
                    ha = max(sa - 1, 0)
                    xtl.append(xtlp.tile([128, KT, sb - ha], F16,
                                         tag=f"xtl{ci}", name=f"xtl{ci}"))

                with tc.tile_pool(name="x8p", bufs=1) as x8p, \
                     tc.tile_pool(name="vhip", bufs=1) as vhip, \
                     tc.tile_pool(name="vlop", bufs=1) as vlop, \
                     tc.tile_pool(name="rinp", bufs=1) as rinp, \
                     tc.tile_pool(name="p1sc", bufs=2) as p1sc, \
                     tc.tile_pool(name="hp", bufs=2) as hp, \
                     tc.tile_pool(name="wvrp", bufs=4) as wvrp, \
                     tc.tile_pool(name="sgp", bufs=2) as sgp, \
                     tc.tile_pool(name="vsbp", bufs=2) as vsbp, \
                     tc.tile_pool(name="oisc", bufs=2) as oisc:
                    vhi = [vhip.tile([128, KT, pb - pa], F8, tag=f"vhi{ci}",
                                     name=f"vhi{ci}")
                           for ci, (pa, pb) in enumerate(CH)]
                    vlo = [vlop.tile([128, KT, pb - pa], F8, tag=f"vlo{ci}",
                                     name=f"vlo{ci}")
                           for ci, (pa, pb) in enumerate(CH)]
                    rin = [rinp.tile([128, KT, pb - pa], F8, tag=f"rin{ci}",
                                     name=f"rin{ci}")
                           for ci, (pa, pb) in enumerate(CH)]
                    x8l = [x8p.tile([128, KT, 342], F8, tag=f"x8l{ci}",
                                    name=f"x8l{ci}") for ci in range(3)]
                    # batched x loads: one DMA per chunk per dtype
                    for ci, (sa, sb) in enumerate(ST1):
                        ha = max(sa - 1, 0)
                        nc.sync.dma_start(x8l[ci][:], xT8[ci])
                        nc.sync.dma_start(xtl[ci][:], xT[:, :, ha:sb])
                    ln1 = {}

                    def ln1_rowsbc(ci, prev):
                        sa, sb = ST1[ci]
                        ha = max(sa - 1, 0)
                        hn = sb - ha
                        n = sb - sa
                        a_rc, c_rc = stats_rows(x8l[ci][:, :, :n], n)
                        ln1[ci] = (bcast2(a_rc, c_rc, n, prev, hn), ha, hn)
                        return (a_rc, c_rc, n)

                    def ln1_apply_ki(ci, ki):
                        (ab, cb), ha, hn = ln1[ci]
                        nmix = hn - 1
                        tt = p1sc.tile([128, 512], F16, tag="tt")
                        nc.vector.tensor_mul(tt[:, :hn], xtl[ci][:, ki, :],
                                             ab[:, :hn])
                        h = hp.tile([128, 512], F16, tag="h")
                        nc.vector.tensor_add(h[:, :hn], tt[:, :hn],
                                             cb[:, :hn])
                        d = p1sc.tile([128, 512], F16, tag="d")
                        nc.vector.tensor_sub(d[:, :nmix], h[:, 1:hn],
                                             h[:, :nmix])
                        v16 = p1sc.tile([128, 512], F16, tag="v16")
                        nc.vector.scalar_tensor_tensor(
                            v16[:, :nmix], d[:, :nmix],
                            mv[:, ki : ki + 1], h[:, :nmix],
                            op0=OP.mult, op1=OP.add)
                        nc.vector.scalar_tensor_tensor(
                            rin[ci][:, ki, :], d[:, :nmix],
                            mr[:, ki : ki + 1], h[:, :nmix],
                            op0=OP.mult, op1=OP.add)
                        nc.scalar.copy(vhi[ci][:, ki, :], v16[:, :nmix])
                        nc.gpsimd.tensor_sub(vlo[ci][:, ki, :],
                                             v16[:, :nmix],
                                             vhi[ci][:, ki, :])

                    def mm_vr(ci, extra=None):
                        pa, pb = CH[ci]
                        n = pb - pa
                        for oi in range(OT):
                            if extra is not None:
                                extra(oi)
                            wt = wvrp.tile([128, 3, KT, 128], F8, tag="wvr")
                            nc.sync.dma_start(wt[:], Wvr[oi])
                            vps = mm.tile([128, 512], F32, tag="acc")
                            for ki in range(0, KT, 2):
                                nc.tensor.matmul(
                                    vps[:, :n], wt[:, 0, ki : ki + 2, :],
                                    vhi[ci][:, ki : ki + 2, :],
                                    start=(ki == 0), stop=False, perf_mode=DR)
                            for ki in range(0, KT, 2):
                                nc.tensor.matmul(
                                    vps[:, :n], wt[:, 1, ki : ki + 2, :],
                                    vhi[ci][:, ki : ki + 2, :],
                                    start=False, stop=False, perf_mode=DR)
                            for ki in range(0, KT, 2):
                                nc.tensor.matmul(
                                    vps[:, :n], wt[:, 0, ki : ki + 2, :],
                                    vlo[ci][:, ki : ki + 2, :],
                                    start=False, stop=(ki == KT - 2),
                                    perf_mode=DR)
                            rps = mm.tile([128, 512], F32, tag="acc")
                            for ki in range(0, KT, 2):
                                nc.tensor.matmul(
                                    rps[:, :n], wt[:, 2, ki : ki + 2, :],
                                    rin[ci][:, ki : ki + 2, :],
                                    start=(ki == 0), stop=(ki == KT - 2),
                                    perf_mode=DR)
                            sg = sgp.tile([128, 512], F16, tag="sg")
                            nc.scalar.activation(sg[:, :n], rps[:, :n],
                                                 AF.Sigmoid, scale=ISW)
                            vsb = vsbp.tile([128, 512], F16, tag="vsb")
                            nc.scalar.activation(vsb[:, :n], vps[:, :n],
                                                 AF.Copy, scale=ISW)
                            oi16 = oisc.tile([128, 512], F16, tag="oi16")
                            nc.vector.tensor_mul(oi16[:, :n], sg[:, :n],
                                                 vsb[:, :n])
                            nc.scalar.copy(ohi[ci][:, oi, :], oi16[:, :n])
                            nc.gpsimd.tensor_sub(olo[ci][:, oi, :],
                                                 oi16[:, :n],
                                                 ohi[ci][:, oi, :])

                    pr = ln1_rowsbc(0, None)
                    for ki in range(KT):
                        ln1_apply_ki(0, ki)
                    pr1 = ln1_rowsbc(1, pr)
                    mm_vr(0, extra=lambda oi: ln1_apply_ki(1, oi))
                    ln1_rowsbc(2, pr1)
                    mm_vr(1, extra=lambda oi: ln1_apply_ki(2, oi))
                    mm_vr(2)

                # ---------- o-proj + residual + LN2 ----------
                with tc.tile_pool(name="x2b", bufs=1) as x2bp, \
                     tc.tile_pool(name="x2b8", bufs=1) as x2b8p, \
                     tc.tile_pool(name="wop", bufs=6) as wop, \
                     tc.tile_pool(name="dsc", bufs=3) as dsc, \
                     tc.tile_pool(name="h2p", bufs=2) as h2p, \
                     tc.tile_pool(name="cyp", bufs=1) as cyp:
                    x2b = {}
                    x2b8 = {}
                    ln2 = {}
                    carry = {}
                    for ci in range(3):
                        x2b[ci] = x2bp.tile([128, KT, 342], F16,
                                            tag=f"x2b{ci}", name=f"x2b{ci}")
                        x2b8[ci] = x2b8p.tile([128, KT, 342], F8,
                                              tag=f"x2b8{ci}", name=f"x2b8{ci}")
                        carry[ci] = cyp.tile([128, KT], F16, tag=f"cy{ci}",
                                             name=f"cy{ci}")

                    def c_step(ci, oi):
                        pa, pb = CH[ci]
                        sa, sb = ST1[ci]
                        ha = max(sa - 1, 0)
                        n = pb - pa
                        wt = wop.tile([128, 2, KT, 128], F8, tag="wo")
                        nc.sync.dma_start(wt[:], Woc[oi])
                        ops_ = mm.tile([128, 512], F32, tag="acc")
                        for ki in range(0, KT, 2):
                            nc.tensor.matmul(
                                ops_[:, :n], wt[:, 0, ki : ki + 2, :],
                                ohi[ci][:, ki : ki + 2, :],
                                start=(ki == 0), stop=False, perf_mode=DR)
                        for ki in range(0, KT, 2):
                            nc.tensor.matmul(
                                ops_[:, :n], wt[:, 1, ki : ki + 2, :],
                                ohi[ci][:, ki : ki + 2, :],
                                start=False, stop=False, perf_mode=DR)
                        for ki in range(0, KT, 2):
                            nc.tensor.matmul(
                                ops_[:, :n], wt[:, 0, ki : ki + 2, :],
                                olo[ci][:, ki : ki + 2, :],
                                start=False, stop=(ki == KT - 2), perf_mode=DR)
                        nc.vector.scalar_tensor_tensor(
                            x2b[ci][:, oi, :n], ops_[:, :n], ISW,
                            xtl[ci][:, oi, pa - ha : pb - ha],
                            op0=OP.mult, op1=OP.add)
                        nc.scalar.copy(x2b8[ci][:, oi, :n], x2b[ci][:, oi, :n])

                    def spill_x1f(ci):
                        pa, pb = CH[ci]
                        n = pb - pa
                        nc.sync.dma_start(
                            x1f[:, :, pa:pb].transpose([1, 0, 2]),
                            x2b[ci][:, :, :n])

                    def d_rowsbc(ci):
                        pa, pb = CH[ci]
                        n = pb - pa
                        a_rc, c_rc = stats_rows(x2b8[ci][:, :, :n], n)
                        ln2[ci] = bcast2(a_rc, c_rc, n, None, n)

                    def d_apply_ki(ci, ki):
                        pa, pb = CH[ci]
                        n = pb - pa
                        ab, cb = ln2[ci]
                        tt = dsc.tile([128, 512], F16, tag="tt2")
                        nc.gpsimd.tensor_mul(tt[:, :n], x2b[ci][:, ki, :n],
                                             ab[:, :n])
                        h2 = h2p.tile([128, 512], F16, tag="h2")
                        nc.vector.tensor_add(h2[:, :n], tt[:, :n], cb[:, :n])
                        if ci > 0:
                            pcy = carry[ci - 1]
                            db = dsc.tile([128, 1], F16, tag="db")
                            nc.vector.tensor_sub(
                                db[:], h2[:, 0:1], pcy[:, ki : ki + 1])
                            gidx = pa - 2
                            nc.vector.scalar_tensor_tensor(
                                cmt[:, ki, gidx : gidx + 1], db[:],
                                mk[:, ki : ki + 1], pcy[:, ki : ki + 1],
                                op0=OP.mult, op1=OP.add)
                        d2 = dsc.tile([128, 512], F16, tag="d2")
                        nc.vector.tensor_sub(d2[:, : n - 1], h2[:, 1:n],
                                             h2[:, : n - 1])
                        glo, ghi = pa - 1, pb - 2
                        nc.vector.scalar_tensor_tensor(
                            cmt[:, ki, glo:ghi], d2[:, : ghi - glo],
                            mk[:, ki : ki + 1], h2[:, : ghi - glo],
                            op0=OP.mult, op1=OP.add)
                        nc.vector.tensor_copy(carry[ci][:, ki : ki + 1],
                                              h2[:, n - 1 : n])

                    for oi in range(OT):
                        c_step(0, oi)
                    spill_x1f(0)
                    d_rowsbc(0)
                    for oi in range(OT):
                        c_step(1, oi)
                        d_apply_ki(0, oi)
                    spill_x1f(1)
                    d_rowsbc(1)
                    for oi in range(OT):
                        c_step(2, oi)
                        d_apply_ki(1, oi)
                    spill_x1f(2)
                    d_rowsbc(2)
                    for ki in range(KT):
                        d_apply_ki(2, ki)

            # ---------- FFN single-pass ----------
            psg.close()
            with tc.tile_pool(name="mme", bufs=8, space="PSUM") as mme, \
                 tc.tile_pool(name="wkp", bufs=2) as wkp, \
                 tc.tile_pool(name="wvcp", bufs=3) as wvcp, \
                 tc.tile_pool(name="silup", bufs=1) as silup, \
                 tc.tile_pool(name="fsc", bufs=2) as fsc, \
                 tc.tile_pool(name="prp", bufs=2) as prp:
                sil = silup.tile([128, UPT, 1024], F8)

                def key_chain(wk_g, gi, ui, hf):
                    cs = hf * 512
                    kps = mme.tile([128, 512], F32, tag="acc")
                    for ki in range(0, KT, 2):
                        nc.tensor.matmul(
                            kps[:], wk_g[:, ui - gi * KG, ki : ki + 2, :],
                            cmt[:, ki : ki + 2, cs : cs + 512],
                            start=(ki == 0), stop=(ki == KT - 2), perf_mode=DR)
                    nc.scalar.activation(sil[:, ui, cs : cs + 512], kps[:],
                                         AF.Silu, scale=ISW)

                for gi in range(UPT // KG):
                    wk_g = wkp.tile([128, KG, KT, 128], F8, tag="wkg")
                    nc.sync.dma_start(wk_g[:], Wkey8[gi * KG : (gi + 1) * KG]
                                      .transpose([1, 0, 2, 3]))
                    for ui in range(gi * KG, (gi + 1) * KG):
                        key_chain(wk_g, gi, ui, 0)
                    for ui in range(gi * KG, (gi + 1) * KG):
                        key_chain(wk_g, gi, ui, 1)

                for oi in range(OT):
                    wt = wvcp.tile([128, UPT + KT, 128], F8, tag="wvc")
                    nc.sync.dma_start(wt[:], Wvc[oi])
                    x1t = fsc.tile([128, 1024], F16, tag="x1r")
                    nc.sync.dma_start(x1t[:], x1f[oi, :, 2 : 2 + 1024])
                    prod = prp.tile([128, 1024], F32, tag="prod")
                    for hf in range(2):
                        cs = hf * 512
                        kvps = mme.tile([128, 512], F32, tag="acc")
                        for ki in range(0, UPT, 2):
                            nc.tensor.matmul(
                                kvps[:], wt[:, ki : ki + 2, :],
                                sil[:, ki : ki + 2, cs : cs + 512],
                                start=(ki == 0), stop=(ki == UPT - 2),
                                perf_mode=DR)
                        rrps = mme.tile([128, 512], F32, tag="acc")
                        for ki in range(0, KT, 2):
                            nc.tensor.matmul(
                                rrps[:], wt[:, UPT + ki : UPT + ki + 2, :],
                                cmt[:, ki : ki + 2, cs : cs + 512],
                                start=(ki == 0), stop=(ki == KT - 2),
                                perf_mode=DR)
                        sr = fsc.tile([128, 512], F16, tag="sr")
                        nc.scalar.activation(sr[:], rrps[:], AF.Sigmoid,
                                             scale=ISW)
                        nc.vector.scalar_tensor_tensor(
                            prod[:, cs : cs + 512], kvps[:], ISWV, sr[:],
                            op0=OP.mult, op1=OP.mult)
                        nc.vector.tensor_add(prod[:, cs : cs + 512],
                                             prod[:, cs : cs + 512],
                                             x1t[:, cs : cs + 512])
                    nc.sync.dma_start(out[oi], prod[:])
    nc.compile()
    return nc


def get_nc():
    if "nc" not in _BUILD_CACHE:
        _BUILD_CACHE["nc"] = build()
    return _BUILD_CACHE["nc"]


def make_in_maps(inputs):
    x = np.asarray(inputs["x"], dtype=np.float32)
    Wv8, Wvlo = _p8_hilo(inputs["Wv"], SW)
    Wo8, Wolo = _p8_hilo(inputs["Wo"], SW)
    Wr8 = _p8(inputs["Wr"], SW)
    # combined panels: [OT, 128, parts, KT, 128]
    Wvr = np.ascontiguousarray(np.stack([Wv8, Wvlo, Wr8], axis=2))
    Woc = np.ascontiguousarray(np.stack([Wo8, Wolo], axis=2))
    Wval8 = _p8(inputs["Wval"], SWV)         # [OT, 128, UPT, 128]
    Wcr8 = _p8(inputs["Wcr"], SW)            # [OT, 128, KT, 128]
    Wvc = np.ascontiguousarray(np.concatenate([Wval8, Wcr8], axis=2))
    shared = {
        "Wvr": Wvr, "Woc": Woc,
        "Wkey8": np.ascontiguousarray(_p8(inputs["Wkey"], SW)),
        "Wvc": Wvc,
        "mixa": np.ascontiguousarray(np.stack(
            [_mix128(inputs["tm_mv"]), _mix128(inputs["tm_mr"]),
             _mix128(inputs["cm_mk"])], axis=1)),
    }
    in_maps = []
    for c in range(8):
        b, half = divmod(c, 2)
        s = half * 1024
        xs = np.zeros((TCORE, H), np.float32)
        lo = max(s - 2, 0)
        xs[2 - (s - lo):, :] = x[b, lo : s + 1024, :]
        xs16 = xs.T.astype(np.float16)                     # [H, TCORE]
        xp = xs16.reshape(KT, 128, TCORE).transpose(1, 0, 2)
        m = dict(shared)
        m["xT"] = np.ascontiguousarray(xp)
        xp8 = xp.astype(E4)
        m["xT8"] = np.ascontiguousarray(
            np.stack([xp8[:, :, 0:342], xp8[:, :, 342:684],
                      xp8[:, :, 684:1026]]))
        in_maps.append(m)
    return in_maps


def run(inputs, **kw):
    from concourse.bass_utils import run_bass_kernel_spmd

    in_maps = make_in_maps(inputs)
    nc = get_nc()
    res = run_bass_kernel_spmd(nc, in_maps, core_ids=list(range(8)), **kw)
    outa = np.empty((B, T, H), np.float32)
    for c in range(8):
        b, half = divmod(c, 2)
        o = res.results[c]["out"].reshape(H, 1024)
        outa[b, half * 1024 : (half + 1) * 1024, :] = o.T
    return outa, res


def kernel(**inputs):
    return run(inputs)[0]



# revision 4
# speedup vs baseline: 1.0211x; 1.0211x over previous
"""RWKV GPT block kernel for 8 Trainium2 NeuronCores — fp8 DoubleRow v3.

Same quantization plan as v2 (r/cr/key/val plain fp8 DoubleRow; v/o 3-term
hi/lo fp8; LN stats via fp8 ones-matmuls), plus scheduling fixes:

- Batched DMAs: x loaded with one DMA per chunk per dtype ([128, KT, T]
  DRAM layout); per-oi weight panels combined into single tensors
  (Wvr = v-hi|v-lo|r, Woc = o-hi|o-lo, Wvc = val|cr) so each oi costs one
  descriptor; x1f spilled with one transposed-AP DMA per chunk.
- LN1 applies interleaved between per-chunk matmul sections so the DVE
  apply work of chunk i+1 hides under chunk i's PE chains.
- vlo/olo residual subs on GpSimd to unload the DVE.
- FFN: key matmuls grouped (8 uis per weight-hold) with the half-0 pass
  first so the group-0 half-0 chains (which only need LN2 chunks 0/1)
  cover the chunk-2 LN2 applies.
"""
import sys

sys.path.insert(0, "/opt/trn_rl_repo")
sys.path.insert(0, "/opt/pypackages")

import numpy as np
import ml_dtypes

H = 2048
KT = H // 128
OT = H // 128
UPT = 4 * H // 128
B = 4
T = 2048
TCORE = 1026
EPS = 1e-5
INV_H = 1.0 / H
SW = 32.0
SWV = 64.0
ISW = 1.0 / SW
ISWV = 1.0 / SWV
E4 = ml_dtypes.float8_e4m3

CH = [(1, 342), (342, 684), (684, 1026)]
ST1 = [(0, 342), (342, 684), (684, 1026)]
KG = 8                 # FFN key-group size (uis per weight hold)

_BUILD_CACHE = {}


def _panels(WT):
    IN, OUT = WT.shape
    kt, ot = IN // 128, OUT // 128
    return np.ascontiguousarray(
        WT.reshape(kt, 128, ot, 128).transpose(2, 1, 0, 3))


def _p8(W, scale):
    a = _panels(np.asarray(W, np.float32).T) * scale
    return a.astype(E4)


def _p8_hilo(W, scale):
    a = _panels(np.asarray(W, np.float32).T) * scale
    hi = a.astype(E4)
    lo = (a - hi.astype(np.float32)).astype(E4)
    return hi, lo


def _mix128(v):
    return np.ascontiguousarray(
        np.asarray(v, dtype=np.float32).reshape(-1)[:H].reshape(KT, 128).T)


def build():
    import contextlib

    import concourse.bacc as bacc
    import concourse.mybir as mybir
    import concourse.tile as tile

    F16 = mybir.dt.float16
    F32 = mybir.dt.float32
    F8 = mybir.dt.float8e4
    AF = mybir.ActivationFunctionType
    OP = mybir.AluOpType
    DR = mybir.MatmulPerfMode.DoubleRow

    nc = bacc.Bacc("TRN2", target_bir_lowering=False)

    xT = nc.dram_tensor("xT", [128, KT, TCORE], F16, kind="ExternalInput")
    xT8 = nc.dram_tensor("xT8", [3, 128, KT, 342], F8, kind="ExternalInput")
    Wvr = nc.dram_tensor("Wvr", [OT, 128, 2, KT, 128], F8, kind="ExternalInput")
    Woc = nc.dram_tensor("Woc", [OT, 128, 2, KT, 128], F8, kind="ExternalInput")
    Wkey8 = nc.dram_tensor("Wkey8", [UPT, 128, KT, 128], F8, kind="ExternalInput")
    Wvc = nc.dram_tensor("Wvc", [OT, 128, UPT + KT, 128], F8,
                         kind="ExternalInput")
    mixa = nc.dram_tensor("mixa", [128, 3, KT], F32, kind="ExternalInput")
    out = nc.dram_tensor("out", [OT, 128, 1024], F32, kind="ExternalOutput")
    x1f = nc.dram_tensor("x1f", [OT, 128, TCORE], F16, kind="Internal")

    with tile.TileContext(nc) as tc, contextlib.ExitStack() as g:
        cpool = g.enter_context(tc.tile_pool(name="consts", bufs=1))
        psg = contextlib.ExitStack()
        st = psg.enter_context(tc.tile_pool(name="st", bufs=1, space="PSUM"))
        mm = psg.enter_context(tc.tile_pool(name="mm", bufs=6, space="PSUM"))
        rows = g.enter_context(tc.tile_pool(name="rows", bufs=2))
        rsc = g.enter_context(tc.tile_pool(name="rsc", bufs=1))
        bcs = g.enter_context(tc.tile_pool(name="bcs", bufs=2))
        sqp = g.enter_context(tc.tile_pool(name="sqp", bufs=1))

        ones_r = cpool.tile([1, 128], F16)
        nc.vector.memset(ones_r[:], 1.0)
        ones8 = cpool.tile([128, 2, 128], F8)
        nc.vector.memset(ones8[:], 1.0)
        mixt = cpool.tile([128, 3, KT], F32)
        nc.sync.dma_start(mixt[:], mixa[:])
        mv = mixt[:, 0]
        mr = mixt[:, 1]
        mk = mixt[:, 2]

        def stats_rows(src8, n):
            s1 = st.tile([128, 512], F32, tag="s1")
            s2 = st.tile([128, 512], F32, tag="s2")
            sq = sqp.tile([128, KT, 342], F8, tag="sq")
            for ki in range(KT):
                nc.scalar.square(sq[:, ki, :n], src8[:, ki, :])
            for ki in range(0, KT, 2):
                nc.tensor.matmul(s1[:, :n], ones8[:], src8[:, ki : ki + 2, :],
                                 start=(ki == 0), stop=(ki == KT - 2),
                                 perf_mode=DR)
            for ki in range(0, KT, 2):
                nc.tensor.matmul(s2[:, :n], ones8[:], sq[:, ki : ki + 2, :n],
                                 start=(ki == 0), stop=(ki == KT - 2),
                                 perf_mode=DR)
            m = rsc.tile([1, 512], F32, tag="m")
            nc.vector.tensor_scalar_mul(m[:, :n], s1[0:1, :n], INV_H)
            var = rsc.tile([1, 512], F32, tag="var")
            nc.vector.tensor_scalar_mul(var[:, :n], s2[0:1, :n], INV_H)
            msq = rsc.tile([1, 512], F32, tag="msd")
            nc.vector.tensor_mul(msq[:, :n], m[:, :n], m[:, :n])
            nc.vector.tensor_sub(var[:, :n], var[:, :n], msq[:, :n])
            nc.vector.tensor_scalar_add(var[:, :n], var[:, :n], EPS)
            sd = rsc.tile([1, 512], F32, tag="msd")
            nc.scalar.sqrt(sd[:, :n], var[:, :n])
            a_rf = rsc.tile([1, 512], F32, tag="var")
            nc.vector.reciprocal(a_rf[:, :n], sd[:, :n])
            a_rc = rows.tile([1, 512], F16, tag="arow")
            nc.vector.tensor_copy(a_rc[:, :n], a_rf[:, :n])
            c_rc = rows.tile([1, 512], F16, tag="crow")
            nc.vector.scalar_tensor_tensor(
                c_rc[:, :n], m[:, :n], -1.0, a_rf[:, :n],
                op0=OP.mult, op1=OP.mult)
            return a_rc, c_rc

        def bcast2(a_rc, c_rc, n, prev, hn):
            off = hn - n
            abp = st.tile([128, 512], F32, tag="s1")
            cbp = st.tile([128, 512], F32, tag="s2")
            if off:
                pa_rc, pc_rc, pn = prev
                nc.tensor.matmul(abp[:, 0:1], ones_r[:], pa_rc[:, pn - 1 : pn],
                                 start=True, stop=True, skip_group_check=True)
                nc.tensor.matmul(cbp[:, 0:1], ones_r[:], pc_rc[:, pn - 1 : pn],
                                 start=True, stop=True, skip_group_check=True)
            nc.tensor.matmul(abp[:, off : off + n], ones_r[:], a_rc[:, :n],
                             start=True, stop=True, skip_group_check=True)
            nc.tensor.matmul(cbp[:, off : off + n], ones_r[:], c_rc[:, :n],
                             start=True, stop=True, skip_group_check=True)
            ab = bcs.tile([128, 512], F16, tag="ab")
            nc.scalar.copy(ab[:, :hn], abp[:, :hn])
            cb = bcs.tile([128, 512], F16, tag="cb")
            nc.scalar.copy(cb[:, :hn], cbp[:, :hn])
            return ab, cb

        with tc.tile_pool(name="cmp", bufs=1) as cmp_:
            cmt = cmp_.tile([128, KT, 1024], F8)

            with tc.tile_pool(name="ohip", bufs=1) as ohip, \
                 tc.tile_pool(name="olop", bufs=1) as olop, \
                 tc.tile_pool(name="xtlp", bufs=1) as xtlp:
                ohi = [ohip.tile([128, KT, pb - pa], F8, tag=f"ohi{ci}",
                                 name=f"ohi{ci}")
                       for ci, (pa, pb) in enumerate(CH)]
                olo = [olop.tile([128, KT, pb - pa], F8, tag=f"olo{ci}",
                                 name=f"olo{ci}")
                       for ci, (pa, pb) in enumerate(CH)]
                xtl = []
                for ci, (sa, sb) in enumerate(ST1):
                    ha = max(sa - 1, 0)
                    xtl.append(xtlp.tile([128, KT, sb - ha], F16,
                                         tag=f"xtl{ci}", name=f"xtl{ci}"))

                with tc.tile_pool(name="x8p", bufs=1) as x8p, \
                     tc.tile_pool(name="vhip", bufs=1) as vhip, \
                     tc.tile_pool(name="vlop", bufs=1) as vlop, \
                     tc.tile_pool(name="rinp", bufs=1) as rinp, \
                     tc.tile_pool(name="p1sc", bufs=2) as p1sc, \
                     tc.tile_pool(name="hp", bufs=2) as hp, \
                     tc.tile_pool(name="wvrp", bufs=4) as wvrp, \
                     tc.tile_pool(name="sgp", bufs=2) as sgp, \
                     tc.tile_pool(name="vsbp", bufs=2) as vsbp, \
                     tc.tile_pool(name="oisc", bufs=2) as oisc:
                    vhi = [vhip.tile([128, KT, pb - pa], F8, tag=f"vhi{ci}",
                                     name=f"vhi{ci}")
                           for ci, (pa, pb) in enumerate(CH)]
                    vlo = [vlop.tile([128, KT, pb - pa], F8, tag=f"vlo{ci}",
                                     name=f"vlo{ci}")
                           for ci, (pa, pb) in enumerate(CH)]
                    rin = [rinp.tile([128, KT, pb - pa], F8, tag=f"rin{ci}",
                                     name=f"rin{ci}")
                           for ci, (pa, pb) in enumerate(CH)]
                    x8l = [x8p.tile([128, KT, 342], F8, tag=f"x8l{ci}",
                                    name=f"x8l{ci}") for ci in range(3)]
                    # batched x loads: one DMA per chunk per dtype
                    for ci, (sa, sb) in enumerate(ST1):
                        ha = max(sa - 1, 0)
                        nc.sync.dma_start(x8l[ci][:], xT8[ci])
                        nc.sync.dma_start(xtl[ci][:], xT[:, :, ha:sb])
                    ln1 = {}

                    def ln1_rowsbc(ci, prev):
                        sa, sb = ST1[ci]
                        ha = max(sa - 1, 0)
                        hn = sb - ha
                        n = sb - sa
                        a_rc, c_rc = stats_rows(x8l[ci][:, :, :n], n)
                        ln1[ci] = (bcast2(a_rc, c_rc, n, prev, hn), ha, hn)
                        return (a_rc, c_rc, n)

                    def ln1_apply_ki(ci, ki):
                        (ab, cb), ha, hn = ln1[ci]
                        nmix = hn - 1
                        tt = p1sc.tile([128, 512], F16, tag="tt")
                        nc.vector.tensor_mul(tt[:, :hn], xtl[ci][:, ki, :],
                                             ab[:, :hn])
                        h = hp.tile([128, 512], F16, tag="h")
                        nc.vector.tensor_add(h[:, :hn], tt[:, :hn],
                                             cb[:, :hn])
                        d = p1sc.tile([128, 512], F16, tag="d")
                        nc.vector.tensor_sub(d[:, :nmix], h[:, 1:hn],
                                             h[:, :nmix])
                        v16 = p1sc.tile([128, 512], F16, tag="v16")
                        nc.vector.scalar_tensor_tensor(
                            v16[:, :nmix], d[:, :nmix],
                            mv[:, ki : ki + 1], h[:, :nmix],
                            op0=OP.mult, op1=OP.add)
                        nc.vector.scalar_tensor_tensor(
                            rin[ci][:, ki, :], d[:, :nmix],
                            mr[:, ki : ki + 1], h[:, :nmix],
                            op0=OP.mult, op1=OP.add)
                        nc.scalar.copy(vhi[ci][:, ki, :], v16[:, :nmix])
                        nc.gpsimd.tensor_sub(vlo[ci][:, ki, :],
                                             v16[:, :nmix],
                                             vhi[ci][:, ki, :])

                    def mm_vr(ci, extra=None):
                        pa, pb = CH[ci]
                        n = pb - pa
                        for oi in range(OT):
                            if extra is not None:
                                extra(oi)
                            wt = wvrp.tile([128, 2, KT, 128], F8, tag="wvr")
                            nc.sync.dma_start(wt[:], Wvr[oi])
                            vps = mm.tile([128, 512], F32, tag="acc")
                            for ki in range(0, KT, 2):
                                nc.tensor.matmul(
                                    vps[:, :n], wt[:, 0, ki : ki + 2, :],
                                    vhi[ci][:, ki : ki + 2, :],
                                    start=(ki == 0), stop=False, perf_mode=DR)
                            for ki in range(0, KT, 2):
                                nc.tensor.matmul(
                                    vps[:, :n], wt[:, 0, ki : ki + 2, :],
                                    vlo[ci][:, ki : ki + 2, :],
                                    start=False, stop=(ki == KT - 2),
                                    perf_mode=DR)
                            rps = mm.tile([128, 512], F32, tag="acc")
                            for ki in range(0, KT, 2):
                                nc.tensor.matmul(
                                    rps[:, :n], wt[:, 1, ki : ki + 2, :],
                                    rin[ci][:, ki : ki + 2, :],
                                    start=(ki == 0), stop=(ki == KT - 2),
                                    perf_mode=DR)
                            sg = sgp.tile([128, 512], F16, tag="sg")
                            nc.scalar.activation(sg[:, :n], rps[:, :n],
                                                 AF.Sigmoid, scale=ISW)
                            vsb = vsbp.tile([128, 512], F16, tag="vsb")
                            nc.scalar.activation(vsb[:, :n], vps[:, :n],
                                                 AF.Copy, scale=ISW)
                            oi16 = oisc.tile([128, 512], F16, tag="oi16")
                            nc.vector.tensor_mul(oi16[:, :n], sg[:, :n],
                                                 vsb[:, :n])
                            nc.scalar.copy(ohi[ci][:, oi, :], oi16[:, :n])
                            nc.gpsimd.tensor_sub(olo[ci][:, oi, :],
                                                 oi16[:, :n],
                                                 ohi[ci][:, oi, :])

                    pr = ln1_rowsbc(0, None)
                    for ki in range(KT):
                        ln1_apply_ki(0, ki)
                    pr1 = ln1_rowsbc(1, pr)
                    mm_vr(0, extra=lambda oi: ln1_apply_ki(1, oi))
                    ln1_rowsbc(2, pr1)
                    mm_vr(1, extra=lambda oi: ln1_apply_ki(2, oi))
                    mm_vr(2)

                # ---------- o-proj + residual + LN2 ----------
                with tc.tile_pool(name="x2b", bufs=1) as x2bp, \
                     tc.tile_pool(name="x2b8", bufs=1) as x2b8p, \
                     tc.tile_pool(name="wop", bufs=6) as wop, \
                     tc.tile_pool(name="dsc", bufs=3) as dsc, \
                     tc.tile_pool(name="h2p", bufs=2) as h2p, \
                     tc.tile_pool(name="cyp", bufs=1) as cyp:
                    x2b = {}
                    x2b8 = {}
                    ln2 = {}
                    carry = {}
                    for ci in range(3):
                        x2b[ci] = x2bp.tile([128, KT, 342], F16,
                                            tag=f"x2b{ci}", name=f"x2b{ci}")
                        x2b8[ci] = x2b8p.tile([128, KT, 342], F8,
                                              tag=f"x2b8{ci}", name=f"x2b8{ci}")
                        carry[ci] = cyp.tile([128, KT], F16, tag=f"cy{ci}",
                                             name=f"cy{ci}")

                    def c_step(ci, oi):
                        pa, pb = CH[ci]
                        sa, sb = ST1[ci]
                        ha = max(sa - 1, 0)
                        n = pb - pa
                        wt = wop.tile([128, 2, KT, 128], F8, tag="wo")
                        nc.sync.dma_start(wt[:], Woc[oi])
                        ops_ = mm.tile([128, 512], F32, tag="acc")
                        for ki in range(0, KT, 2):
                            nc.tensor.matmul(
                                ops_[:, :n], wt[:, 0, ki : ki + 2, :],
                                ohi[ci][:, ki : ki + 2, :],
                                start=(ki == 0), stop=False, perf_mode=DR)
                        for ki in range(0, KT, 2):
                            nc.tensor.matmul(
                                ops_[:, :n], wt[:, 1, ki : ki + 2, :],
                                ohi[ci][:, ki : ki + 2, :],
                                start=False, stop=False, perf_mode=DR)
                        for ki in range(0, KT, 2):
                            nc.tensor.matmul(
                                ops_[:, :n], wt[:, 0, ki : ki + 2, :],
                                olo[ci][:, ki : ki + 2, :],
                                start=False, stop=(ki == KT - 2), perf_mode=DR)
                        nc.vector.scalar_tensor_tensor(
                            x2b[ci][:, oi, :n], ops_[:, :n], ISW,
                            xtl[ci][:, oi, pa - ha : pb - ha],
                            op0=OP.mult, op1=OP.add)
                        nc.scalar.copy(x2b8[ci][:, oi, :n], x2b[ci][:, oi, :n])

                    def spill_x1f(ci):
                        pa, pb = CH[ci]
                        n = pb - pa
                        nc.sync.dma_start(
                            x1f[:, :, pa:pb].transpose([1, 0, 2]),
                            x2b[ci][:, :, :n])

                    def d_rowsbc(ci):
                        pa, pb = CH[ci]
                        n = pb - pa
                        a_rc, c_rc = stats_rows(x2b8[ci][:, :, :n], n)
                        ln2[ci] = bcast2(a_rc, c_rc, n, None, n)

                    def d_apply_ki(ci, ki):
                        pa, pb = CH[ci]
                        n = pb - pa
                        ab, cb = ln2[ci]
                        tt = dsc.tile([128, 512], F16, tag="tt2")
                        nc.gpsimd.tensor_mul(tt[:, :n], x2b[ci][:, ki, :n],
                                             ab[:, :n])
                        h2 = h2p.tile([128, 512], F16, tag="h2")
                        nc.vector.tensor_add(h2[:, :n], tt[:, :n], cb[:, :n])
                        if ci > 0:
                            pcy = carry[ci - 1]
                            db = dsc.tile([128, 1], F16, tag="db")
                            nc.vector.tensor_sub(
                                db[:], h2[:, 0:1], pcy[:, ki : ki + 1])
                            gidx = pa - 2
                            nc.vector.scalar_tensor_tensor(
                                cmt[:, ki, gidx : gidx + 1], db[:],
                                mk[:, ki : ki + 1], pcy[:, ki : ki + 1],
                                op0=OP.mult, op1=OP.add)
                        d2 = dsc.tile([128, 512], F16, tag="d2")
                        nc.vector.tensor_sub(d2[:, : n - 1], h2[:, 1:n],
                                             h2[:, : n - 1])
                        glo, ghi = pa - 1, pb - 2
                        nc.vector.scalar_tensor_tensor(
                            cmt[:, ki, glo:ghi], d2[:, : ghi - glo],
                            mk[:, ki : ki + 1], h2[:, : ghi - glo],
                            op0=OP.mult, op1=OP.add)
                        nc.vector.tensor_copy(carry[ci][:, ki : ki + 1],
                                              h2[:, n - 1 : n])

                    for oi in range(OT):
                        c_step(0, oi)
                    spill_x1f(0)
                    d_rowsbc(0)
                    for oi in range(OT):
                        c_step(1, oi)
                        d_apply_ki(0, oi)
                    spill_x1f(1)
                    d_rowsbc(1)
                    for oi in range(OT):
                        c_step(2, oi)
                        d_apply_ki(1, oi)
                    spill_x1f(2)
                    d_rowsbc(2)
                    for ki in range(KT):
                        d_apply_ki(2, ki)

            # ---------- FFN single-pass ----------
            psg.close()
            with tc.tile_pool(name="mme", bufs=8, space="PSUM") as mme, \
                 tc.tile_pool(name="wkp", bufs=2) as wkp, \
                 tc.tile_pool(name="wvcp", bufs=3) as wvcp, \
                 tc.tile_pool(name="silup", bufs=1) as silup, \
                 tc.tile_pool(name="fsc", bufs=2) as fsc, \
                 tc.tile_pool(name="prp", bufs=2) as prp:
                sil = silup.tile([128, UPT, 1024], F8)

                def key_chain(wk_g, gi, ui, hf):
                    cs = hf * 512
                    kps = mme.tile([128, 512], F32, tag="acc")
                    for ki in range(0, KT, 2):
                        nc.tensor.matmul(
                            kps[:], wk_g[:, ui - gi * KG, ki : ki + 2, :],
                            cmt[:, ki : ki + 2, cs : cs + 512],
                            start=(ki == 0), stop=(ki == KT - 2), perf_mode=DR)
                    nc.scalar.activation(sil[:, ui, cs : cs + 512], kps[:],
                                         AF.Silu, scale=ISW)

                for gi in range(UPT // KG):
                    wk_g = wkp.tile([128, KG, KT, 128], F8, tag="wkg")
                    nc.sync.dma_start(wk_g[:], Wkey8[gi * KG : (gi + 1) * KG]
                                      .transpose([1, 0, 2, 3]))
                    for ui in range(gi * KG, (gi + 1) * KG):
                        key_chain(wk_g, gi, ui, 0)
                    for ui in range(gi * KG, (gi + 1) * KG):
                        key_chain(wk_g, gi, ui, 1)

                for oi in range(OT):
                    wt = wvcp.tile([128, UPT + KT, 128], F8, tag="wvc")
                    nc.sync.dma_start(wt[:], Wvc[oi])
                    x1t = fsc.tile([128, 1024], F16, tag="x1r")
                    nc.sync.dma_start(x1t[:], x1f[oi, :, 2 : 2 + 1024])
                    prod = prp.tile([128, 1024], F32, tag="prod")
                    for hf in range(2):
                        cs = hf * 512
                        kvps = mme.tile([128, 512], F32, tag="acc")
                        for ki in range(0, UPT, 2):
                            nc.tensor.matmul(
                                kvps[:], wt[:, ki : ki + 2, :],
                                sil[:, ki : ki + 2, cs : cs + 512],
                                start=(ki == 0), stop=(ki == UPT - 2),
                                perf_mode=DR)
                        rrps = mme.tile([128, 512], F32, tag="acc")
                        for ki in range(0, KT, 2):
                            nc.tensor.matmul(
                                rrps[:], wt[:, UPT + ki : UPT + ki + 2, :],
                                cmt[:, ki : ki + 2, cs : cs + 512],
                                start=(ki == 0), stop=(ki == KT - 2),
                                perf_mode=DR)
                        sr = fsc.tile([128, 512], F16, tag="sr")
                        nc.scalar.activation(sr[:], rrps[:], AF.Sigmoid,
                                             scale=ISW)
                        nc.vector.scalar_tensor_tensor(
                            prod[:, cs : cs + 512], kvps[:], ISWV, sr[:],
                            op0=OP.mult, op1=OP.mult)
                        nc.vector.tensor_add(prod[:, cs : cs + 512],
                                             prod[:, cs : cs + 512],
                                             x1t[:, cs : cs + 512])
                    nc.sync.dma_start(out[oi], prod[:])
    nc.compile()
    return nc


def get_nc():
    if "nc" not in _BUILD_CACHE:
        _BUILD_CACHE["nc"] = build()
    return _BUILD_CACHE["nc"]


def make_in_maps(inputs):
    x = np.asarray(inputs["x"], dtype=np.float32)
    Wv8 = _p8(inputs["Wv"], SW)
    Wo8, Wolo = _p8_hilo(inputs["Wo"], SW)
    Wr8 = _p8(inputs["Wr"], SW)
    # combined panels: [OT, 128, parts, KT, 128]
    Wvr = np.ascontiguousarray(np.stack([Wv8, Wr8], axis=2))
    Woc = np.ascontiguousarray(np.stack([Wo8, Wolo], axis=2))
    Wval8 = _p8(inputs["Wval"], SWV)         # [OT, 128, UPT, 128]
    Wcr8 = _p8(inputs["Wcr"], SW)            # [OT, 128, KT, 128]
    Wvc = np.ascontiguousarray(np.concatenate([Wval8, Wcr8], axis=2))
    shared = {
        "Wvr": Wvr, "Woc": Woc,
        "Wkey8": np.ascontiguousarray(_p8(inputs["Wkey"], SW)),
        "Wvc": Wvc,
        "mixa": np.ascontiguousarray(np.stack(
            [_mix128(inputs["tm_mv"]), _mix128(inputs["tm_mr"]),
             _mix128(inputs["cm_mk"])], axis=1)),
    }
    in_maps = []
    for c in range(8):
        b, half = divmod(c, 2)
        s = half * 1024
        xs = np.zeros((TCORE, H), np.float32)
        lo = max(s - 2, 0)
        xs[2 - (s - lo):, :] = x[b, lo : s + 1024, :]
        xs16 = xs.T.astype(np.float16)                     # [H, TCORE]
        xp = xs16.reshape(KT, 128, TCORE).transpose(1, 0, 2)
        m = dict(shared)
        m["xT"] = np.ascontiguousarray(xp)
        xp8 = xp.astype(E4)
        m["xT8"] = np.ascontiguousarray(
            np.stack([xp8[:, :, 0:342], xp8[:, :, 342:684],
                      xp8[:, :, 684:1026]]))
        in_maps.append(m)
    return in_maps


def run(inputs, **kw):
    from concourse.bass_utils import run_bass_kernel_spmd

    in_maps = make_in_maps(inputs)
    nc = get_nc()
    res = run_bass_kernel_spmd(nc, in_maps, core_ids=list(range(8)), **kw)
    outa = np.empty((B, T, H), np.float32)
    for c in range(8):
        b, half = divmod(c, 2)
        o = res.results[c]["out"].reshape(H, 1024)
        outa[b, half * 1024 : (half + 1) * 1024, :] = o.T
    return outa, res


def kernel(**inputs):
    return run(inputs)[0]



# revision 40
# speedup vs baseline: 1.0434x; 1.0219x over previous
"""RWKV GPT block kernel for 8 Trainium2 NeuronCores — fp8 DoubleRow v4.

Quantization plan: v-proj 2-pass act-side hi/lo (vhi+vlo @ single fp8 Wv
panel), r/key/val/cr plain fp8 DoubleRow, o-proj 3-term hi/lo fp8, LN
stats via fp8 ones-matmuls.  (rel_err ~= 1.73e-2 vs the 2e-2 gate.)

Scheduling (vs v3):
- x2b (post-TM residual) kept resident in SBUF through the FFN; the
  x1f DRAM spill/reload is gone, freeing c-phase DMA bandwidth.
- LN1 applies batched in ki-pairs (mul/add/sub on [128,2*hn] with
  stride-0 broadcast rows); vhi/vlo conversion batched per pair;
  o-epilogue ohi/olo conversions batched per oi-pair.  This fits the
  DVE/Act/Pool work under the PE pace in the mm_vr phases.
- Startup DMA order: chunk-0 x loads first, later chunks' x loads
  injected mid-phase so TM weight panels stream early.
- FFN key matmuls in 4-ui groups; first 5 groups run their half-0
  chains back-to-back (covering the chunk-2 LN2 apply tail), group 0/1
  weight loads prefetched during the last o-proj phase.
"""
import sys

sys.path.insert(0, "/opt/trn_rl_repo")
sys.path.insert(0, "/opt/pypackages")

import numpy as np
import ml_dtypes

H = 2048
KT = H // 128
OT = H // 128
UPT = 4 * H // 128
B = 4
T = 2048
TCORE = 1026
EPS = 1e-5
INV_H = 1.0 / H
SW = 32.0
SWV = 64.0
ISW = 1.0 / SW
ISWV = 1.0 / SWV
E4 = ml_dtypes.float8_e4m3

CH = [(1, 342), (342, 684), (684, 1026)]
ST1 = [(0, 342), (342, 684), (684, 1026)]
KG = 4                 # FFN key-group size (uis per weight hold)
NKG = UPT // KG
KG_HEAD = 5            # groups whose half-0 chains run before any half-1

_BUILD_CACHE = {}


def _panels(WT):
    IN, OUT = WT.shape
    kt, ot = IN // 128, OUT // 128
    return np.ascontiguousarray(
        WT.reshape(kt, 128, ot, 128).transpose(2, 1, 0, 3))


def _p8(W, scale):
    a = _panels(np.asarray(W, np.float32).T) * scale
    return a.astype(E4)


def _p8_hilo(W, scale):
    a = _panels(np.asarray(W, np.float32).T) * scale
    hi = a.astype(E4)
    lo = (a - hi.astype(np.float32)).astype(E4)
    return hi, lo


def _mix128(v):
    return np.ascontiguousarray(
        np.asarray(v, dtype=np.float32).reshape(-1)[:H].reshape(KT, 128).T)


def build():
    import contextlib

    import concourse.bacc as bacc
    import concourse.mybir as mybir
    import concourse.tile as tile

    F16 = mybir.dt.float16
    F32 = mybir.dt.float32
    F8 = mybir.dt.float8e4
    AF = mybir.ActivationFunctionType
    OP = mybir.AluOpType
    DR = mybir.MatmulPerfMode.DoubleRow

    nc = bacc.Bacc("TRN2", target_bir_lowering=False)

    xT = nc.dram_tensor("xT", [128, KT, TCORE], F16, kind="ExternalInput")
    xT8 = nc.dram_tensor("xT8", [3, 128, KT, 342], F8, kind="ExternalInput")
    Wvr = nc.dram_tensor("Wvr", [OT, 128, 2, KT, 128], F8, kind="ExternalInput")
    Woc = nc.dram_tensor("Woc", [OT, 128, 2, KT, 128], F8, kind="ExternalInput")
    Wkey8 = nc.dram_tensor("Wkey8", [UPT, 128, KT, 128], F8, kind="ExternalInput")
    Wvc = nc.dram_tensor("Wvc", [OT, 128, UPT + KT, 128], F8,
                         kind="ExternalInput")
    mixa = nc.dram_tensor("mixa", [128, 3, KT], F32, kind="ExternalInput")
    out = nc.dram_tensor("out", [OT, 128, 1024], F32, kind="ExternalOutput")
    x1f = nc.dram_tensor("x1f", [OT, 128, TCORE], F16, kind="Internal")

    with tile.TileContext(nc) as tc, contextlib.ExitStack() as g:
        cpool = g.enter_context(tc.tile_pool(name="consts", bufs=1))
        psg = contextlib.ExitStack()
        st = psg.enter_context(tc.tile_pool(name="st", bufs=1, space="PSUM"))
        mm = psg.enter_context(tc.tile_pool(name="mm", bufs=6, space="PSUM"))
        rows = g.enter_context(tc.tile_pool(name="rows", bufs=2))
        rsc = g.enter_context(tc.tile_pool(name="rsc", bufs=1))
        bcs = g.enter_context(tc.tile_pool(name="bcs", bufs=2))
        sqp = g.enter_context(tc.tile_pool(name="sqp", bufs=1))
        ones_r = cpool.tile([1, 128], F16)
        nc.vector.memset(ones_r[:], 1.0)
        ones8 = cpool.tile([128, 2, 128], F8)
        nc.vector.memset(ones8[:], 1.0)
        mixt = cpool.tile([128, 3, KT], F32)
        nc.sync.dma_start(mixt[:], mixa[:])
        mv = mixt[:, 0]
        mr = mixt[:, 1]
        mk = mixt[:, 2]

        def stats_rows(src8, n):
            s1 = st.tile([128, 512], F32, tag="s1")
            s2 = st.tile([128, 512], F32, tag="s2")
            sq = sqp.tile([128, KT, 342], F8, tag="sq")
            for ki in range(KT):
                nc.scalar.square(sq[:, ki, :n], src8[:, ki, :])
            for ki in range(0, KT, 2):
                nc.tensor.matmul(s1[:, :n], ones8[:], src8[:, ki : ki + 2, :],
                                 start=(ki == 0), stop=(ki == KT - 2),
                                 perf_mode=DR)
            for ki in range(0, KT, 2):
                nc.tensor.matmul(s2[:, :n], ones8[:], sq[:, ki : ki + 2, :n],
                                 start=(ki == 0), stop=(ki == KT - 2),
                                 perf_mode=DR)
            m = rsc.tile([1, 512], F32, tag="m")
            nc.vector.tensor_scalar_mul(m[:, :n], s1[0:1, :n], INV_H)
            var = rsc.tile([1, 512], F32, tag="var")
            nc.vector.tensor_scalar_mul(var[:, :n], s2[0:1, :n], INV_H)
            msq = rsc.tile([1, 512], F32, tag="msd")
            nc.vector.tensor_mul(msq[:, :n], m[:, :n], m[:, :n])
            nc.vector.tensor_sub(var[:, :n], var[:, :n], msq[:, :n])
            nc.vector.tensor_scalar_add(var[:, :n], var[:, :n], EPS)
            sd = rsc.tile([1, 512], F32, tag="msd")
            nc.scalar.sqrt(sd[:, :n], var[:, :n])
            a_rf = rsc.tile([1, 512], F32, tag="var")
            nc.vector.reciprocal(a_rf[:, :n], sd[:, :n])
            a_rc = rows.tile([1, 512], F16, tag="arow")
            nc.vector.tensor_copy(a_rc[:, :n], a_rf[:, :n])
            c_rc = rows.tile([1, 512], F16, tag="crow")
            nc.vector.scalar_tensor_tensor(
                c_rc[:, :n], m[:, :n], -1.0, a_rf[:, :n],
                op0=OP.mult, op1=OP.mult)
            return a_rc, c_rc

        def bcast2(a_rc, c_rc, n, prev, hn):
            off = hn - n
            abp = st.tile([128, 512], F32, tag="s1")
            cbp = st.tile([128, 512], F32, tag="s2")
            if off:
                pa_rc, pc_rc, pn = prev
                nc.tensor.matmul(abp[:, 0:1], ones_r[:], pa_rc[:, pn - 1 : pn],
                                 start=True, stop=True, skip_group_check=True)
                nc.tensor.matmul(cbp[:, 0:1], ones_r[:], pc_rc[:, pn - 1 : pn],
                                 start=True, stop=True, skip_group_check=True)
            nc.tensor.matmul(abp[:, off : off + n], ones_r[:], a_rc[:, :n],
                             start=True, stop=True, skip_group_check=True)
            nc.tensor.matmul(cbp[:, off : off + n], ones_r[:], c_rc[:, :n],
                             start=True, stop=True, skip_group_check=True)
            ab = bcs.tile([128, 512], F16, tag="ab")
            nc.scalar.copy(ab[:, :hn], abp[:, :hn])
            cb = bcs.tile([128, 512], F16, tag="cb")
            nc.scalar.copy(cb[:, :hn], cbp[:, :hn])
            return ab, cb

        with tc.tile_pool(name="cmp", bufs=1) as cmp_:
            cmt = cmp_.tile([128, KT, 1024], F8)

            # ohi/olo/vhi/vlo are Act/Pool-written only: keeping them first
            # gives the FFN sil tile a 64KB window free of DVE/DMA-written
            # space (cross-engine waits snapshot full engine counts, so any
            # DVE/DMA overlap would stall the silus behind the LN2 tail).
            with tc.tile_pool(name="ohip", bufs=1) as ohip, \
                 tc.tile_pool(name="olop", bufs=1) as olop, \
                 tc.tile_pool(name="vhip", bufs=1) as vhip, \
                 tc.tile_pool(name="vlop", bufs=1) as vlop, \
                 tc.tile_pool(name="xtlp", bufs=1) as xtlp:
                ohi = [ohip.tile([128, KT, pb - pa], F8, tag=f"ohi{ci}",
                                 name=f"ohi{ci}")
                       for ci, (pa, pb) in enumerate(CH)]
                olo = [olop.tile([128, KT, pb - pa], F8, tag=f"olo{ci}",
                                 name=f"olo{ci}")
                       for ci, (pa, pb) in enumerate(CH)]
                vhi = [vhip.tile([128, KT, pb - pa], F8, tag=f"vhi{ci}",
                                 name=f"vhi{ci}")
                       for ci, (pa, pb) in enumerate(CH)]
                vlo = [vlop.tile([128, KT, pb - pa], F8, tag=f"vlo{ci}",
                                 name=f"vlo{ci}")
                       for ci, (pa, pb) in enumerate(CH)]
                xtl = []
                for ci, (sa, sb) in enumerate(ST1):
                    ha = max(sa - 1, 0)
                    xtl.append(xtlp.tile([128, KT, sb - ha], F16,
                                         tag=f"xtl{ci}", name=f"xtl{ci}"))

                with tc.tile_pool(name="x8p", bufs=1) as x8p, \
                     tc.tile_pool(name="rinp", bufs=1) as rinp, \
                     tc.tile_pool(name="p1sc", bufs=2) as p1sc, \
                     tc.tile_pool(name="hp", bufs=2) as hp, \
                     tc.tile_pool(name="wvrp", bufs=5) as wvrp, \
                     tc.tile_pool(name="sgp", bufs=2) as sgp, \
                     tc.tile_pool(name="vsbp", bufs=2) as vsbp, \
                     tc.tile_pool(name="oisc", bufs=2) as oisc:
                    rin = [rinp.tile([128, KT, pb - pa], F8, tag=f"rin{ci}",
                                     name=f"rin{ci}")
                           for ci, (pa, pb) in enumerate(CH)]
                    x8l = [x8p.tile([128, KT, 342], F8, tag=f"x8l{ci}",
                                    name=f"x8l{ci}") for ci in range(3)]

                    def load_x8(ci):
                        nc.sync.dma_start(x8l[ci][:], xT8[ci])

                    def load_xtl(ci):
                        sa, sb = ST1[ci]
                        ha = max(sa - 1, 0)
                        nc.sync.dma_start(xtl[ci][:], xT[:, :, ha:sb])

                    load_x8(0)
                    load_xtl(0)
                    ln1 = {}

                    def ln1_rowsbc(ci, prev):
                        sa, sb = ST1[ci]
                        ha = max(sa - 1, 0)
                        hn = sb - ha
                        n = sb - sa
                        a_rc, c_rc = stats_rows(x8l[ci][:, :, :n], n)
                        ln1[ci] = (bcast2(a_rc, c_rc, n, prev, hn), ha, hn)
                        return (a_rc, c_rc, n)

                    def ln1_apply_pair(ci, kp):
                        """Batched LN1 apply + mix for ki = 2kp, 2kp+1."""
                        (ab, cb), ha, hn = ln1[ci]
                        nmix = hn - 1
                        k0 = 2 * kp
                        ab_b = ab[:, :hn].unsqueeze(1).broadcast_to(
                            [128, 2, hn])
                        cb_b = cb[:, :hn].unsqueeze(1).broadcast_to(
                            [128, 2, hn])
                        tt = p1sc.tile([128, 2, hn], F16, tag="tt")
                        nc.vector.tensor_mul(tt[:], xtl[ci][:, k0 : k0 + 2, :],
                                             ab_b)
                        h = hp.tile([128, 2, hn], F16, tag="h")
                        nc.vector.tensor_add(h[:], tt[:], cb_b)
                        d = p1sc.tile([128, 2, hn], F16, tag="d")
                        hf = h[:].rearrange("p a b -> p (a b)")
                        df = d[:].rearrange("p a b -> p (a b)")
                        nc.vector.tensor_sub(df[:, : 2 * hn - 1],
                                             hf[:, 1 : 2 * hn],
                                             hf[:, : 2 * hn - 1])
                        v16 = p1sc.tile([128, 2, nmix], F16, tag="v16")
                        for j in range(2):
                            ki = k0 + j
                            nc.vector.scalar_tensor_tensor(
                                v16[:, j, :], d[:, j, :nmix],
                                mv[:, ki : ki + 1], h[:, j, :nmix],
                                op0=OP.mult, op1=OP.add)
                            nc.vector.scalar_tensor_tensor(
                                rin[ci][:, ki, :], d[:, j, :nmix],
                                mr[:, ki : ki + 1], h[:, j, :nmix],
                                op0=OP.mult, op1=OP.add)
                        nc.scalar.copy(vhi[ci][:, k0 : k0 + 2, :], v16[:])
                        nc.gpsimd.tensor_sub(vlo[ci][:, k0 : k0 + 2, :],
                                             v16[:], vhi[ci][:, k0 : k0 + 2, :])

                    def mm_vr(ci, extra=None):
                        pa, pb = CH[ci]
                        n = pb - pa
                        oi16 = {}
                        for oi in range(OT):
                            if extra is not None:
                                extra(oi)
                            wt = wvrp.tile([128, 2, KT, 128], F8, tag="wvr")
                            nc.sync.dma_start(wt[:], Wvr[oi])
                            vps = mm.tile([128, 512], F32, tag="acc")
                            for ki in range(0, KT, 2):
                                nc.tensor.matmul(
                                    vps[:, :n], wt[:, 0, ki : ki + 2, :],
                                    vhi[ci][:, ki : ki + 2, :],
                                    start=(ki == 0), stop=False, perf_mode=DR)
                            for ki in range(0, KT, 2):
                                nc.tensor.matmul(
                                    vps[:, :n], wt[:, 0, ki : ki + 2, :],
                                    vlo[ci][:, ki : ki + 2, :],
                                    start=False, stop=(ki == KT - 2),
                                    perf_mode=DR)
                            rps = mm.tile([128, 512], F32, tag="acc")
                            for ki in range(0, KT, 2):
                                nc.tensor.matmul(
                                    rps[:, :n], wt[:, 1, ki : ki + 2, :],
                                    rin[ci][:, ki : ki + 2, :],
                                    start=(ki == 0), stop=(ki == KT - 2),
                                    perf_mode=DR)
                            sg = sgp.tile([128, 512], F16, tag="sg")
                            nc.scalar.activation(sg[:, :n], rps[:, :n],
                                                 AF.Sigmoid, scale=ISW)
                            vsb = vsbp.tile([128, 512], F16, tag="vsb")
                            nc.scalar.activation(vsb[:, :n], vps[:, :n],
                                                 AF.Copy, scale=ISW)
                            if oi % 2 == 0:
                                oi16[oi // 2] = oisc.tile(
                                    [128, 2, n], F16, tag="oi16",
                                    name=f"oi16_{ci}_{oi}")
                            cur = oi16[oi // 2]
                            nc.vector.tensor_mul(cur[:, oi % 2, :],
                                                 sg[:, :n], vsb[:, :n])
                            if oi % 2 == 1:
                                o0 = oi - 1
                                nc.scalar.copy(ohi[ci][:, o0 : o0 + 2, :],
                                               cur[:])
                                nc.gpsimd.tensor_sub(
                                    olo[ci][:, o0 : o0 + 2, :], cur[:],
                                    ohi[ci][:, o0 : o0 + 2, :])

                    pr = ln1_rowsbc(0, None)
                    for kp in range(KT // 2):
                        ln1_apply_pair(0, kp)
                    load_x8(1)
                    pr1 = ln1_rowsbc(1, pr)
                    load_xtl(1)

                    def extra_c1(oi):
                        if oi == 8:
                            load_x8(2)
                        if oi == 12:
                            load_xtl(2)
                        if oi % 2 == 1:
                            ln1_apply_pair(1, oi // 2)

                    mm_vr(0, extra=extra_c1)
                    ln1_rowsbc(2, pr1)
                    mm_vr(1, extra=lambda oi: (
                        ln1_apply_pair(2, oi // 2) if oi % 2 == 1 else None))
                    mm_vr(2)

                # ---------- o-proj + residual + LN2 ----------
                # pool order fixes SBUF address reuse for the FFN pools
                # stacked after these pop: the d_apply scratch (dsc/h2p/cyp,
                # DVE-written until the chunk-2 LN2 tail ends) must land
                # under wvcp (first written in the val phase), never under
                # silup — else the first silu inherits an ~18us wait.
                # address order (low→high): x2b8 (Act-written) and wop (DMA)
                # land under the FFN's wkp/wvcp claims; x2b and the d_apply
                # scratch sit above everything the FFN pools reach, so no FFN
                # write inherits a wait on the LN2 tail or the spills.
                ost = contextlib.ExitStack()
                x2b8p = ost.enter_context(tc.tile_pool(name="x2b8", bufs=2))
                wop = ost.enter_context(tc.tile_pool(name="wop", bufs=4))
                x2bp = ost.enter_context(tc.tile_pool(name="x2b", bufs=1))
                dsc = ost.enter_context(tc.tile_pool(name="dsc", bufs=3))
                h2p = ost.enter_context(tc.tile_pool(name="h2p", bufs=2))
                cyp = ost.enter_context(tc.tile_pool(name="cyp", bufs=1))
                x2b = {}
                x2b8 = {}
                ln2 = {}
                carry = {}
                for ci in range(3):
                    x2b[ci] = x2bp.tile([128, KT, 342], F16,
                                        tag=f"x2b{ci}", name=f"x2b{ci}")
                    carry[ci] = cyp.tile([128, KT], F16, tag=f"cy{ci}",
                                         name=f"cy{ci}")

                def c_step(ci, oi):
                    pa, pb = CH[ci]
                    sa, sb = ST1[ci]
                    ha = max(sa - 1, 0)
                    n = pb - pa
                    if oi == 0:
                        x2b8[ci] = x2b8p.tile([128, KT, 342], F8,
                                              tag="x2b8", name=f"x2b8{ci}")
                    wt = wop.tile([128, 2, KT, 128], F8, tag="wo")
                    nc.sync.dma_start(wt[:], Woc[oi])
                    ops_ = mm.tile([128, 512], F32, tag="acc")
                    for ki in range(0, KT, 2):
                        nc.tensor.matmul(
                            ops_[:, :n], wt[:, 0, ki : ki + 2, :],
                            ohi[ci][:, ki : ki + 2, :],
                            start=(ki == 0), stop=False, perf_mode=DR)
                    for ki in range(0, KT, 2):
                        nc.tensor.matmul(
                            ops_[:, :n], wt[:, 1, ki : ki + 2, :],
                            ohi[ci][:, ki : ki + 2, :],
                            start=False, stop=False, perf_mode=DR)
                    for ki in range(0, KT, 2):
                        nc.tensor.matmul(
                            ops_[:, :n], wt[:, 0, ki : ki + 2, :],
                            olo[ci][:, ki : ki + 2, :],
                            start=False, stop=(ki == KT - 2), perf_mode=DR)
                    nc.vector.scalar_tensor_tensor(
                        x2b[ci][:, oi, :n], ops_[:, :n], ISW,
                        xtl[ci][:, oi, pa - ha : pb - ha],
                        op0=OP.mult, op1=OP.add)
                    nc.scalar.copy(x2b8[ci][:, oi, :n], x2b[ci][:, oi, :n])

                def d_rowsbc(ci):
                    pa, pb = CH[ci]
                    n = pb - pa
                    a_rc, c_rc = stats_rows(x2b8[ci][:, :, :n], n)
                    ln2[ci] = bcast2(a_rc, c_rc, n, None, n)

                def d_apply_pair(ci, kp):
                    """Batched LN2 apply + mix for ki = 2kp, 2kp+1.

                    All on DVE (no gpsimd): keeping the Pool instruction
                    count confined to the TM phase lets the FFN sil-space
                    anti-dependency resolve long before the first silu.
                    """
                    pa, pb = CH[ci]
                    n = pb - pa
                    k0 = 2 * kp
                    ab, cb = ln2[ci]
                    ab_b = ab[:, :n].unsqueeze(1).broadcast_to([128, 2, n])
                    cb_b = cb[:, :n].unsqueeze(1).broadcast_to([128, 2, n])
                    tt = dsc.tile([128, 2, n], F16, tag="tt2")
                    nc.vector.tensor_mul(tt[:], x2b[ci][:, k0 : k0 + 2, :n],
                                         ab_b)
                    h2 = h2p.tile([128, 2, n], F16, tag="h2")
                    nc.vector.tensor_add(h2[:], tt[:], cb_b)
                    if ci > 0:
                        pcy = carry[ci - 1]
                        db = dsc.tile([128, 2], F16, tag="db")
                        nc.vector.tensor_sub(
                            db[:], h2[:, :, 0], pcy[:, k0 : k0 + 2])
                        gidx = pa - 2
                        for j in range(2):
                            ki = k0 + j
                            nc.vector.scalar_tensor_tensor(
                                cmt[:, ki, gidx : gidx + 1], db[:, j : j + 1],
                                mk[:, ki : ki + 1], pcy[:, ki : ki + 1],
                                op0=OP.mult, op1=OP.add)
                    d2 = dsc.tile([128, 2, n], F16, tag="d2")
                    h2f = h2[:].rearrange("p a b -> p (a b)")
                    d2f = d2[:].rearrange("p a b -> p (a b)")
                    nc.vector.tensor_sub(d2f[:, : 2 * n - 1],
                                         h2f[:, 1 : 2 * n],
                                         h2f[:, : 2 * n - 1])
                    glo, ghi = pa - 1, pb - 2
                    for j in range(2):
                        ki = k0 + j
                        nc.vector.scalar_tensor_tensor(
                            cmt[:, ki, glo:ghi], d2[:, j, : ghi - glo],
                            mk[:, ki : ki + 1], h2[:, j, : ghi - glo],
                            op0=OP.mult, op1=OP.add)
                    nc.vector.tensor_copy(carry[ci][:, k0 : k0 + 2],
                                          h2[:, :, n - 1])

                def spill_x1f(ci):
                    pa, pb = CH[ci]
                    n = pb - pa
                    nc.sync.dma_start(
                        x1f[:, :, pa:pb].transpose([1, 0, 2]),
                        x2b[ci][:, :, :n])

                for oi in range(OT):
                    c_step(0, oi)
                spill_x1f(0)
                d_rowsbc(0)
                for oi in range(OT):
                    c_step(1, oi)
                    if oi % 2 == 1:
                        d_apply_pair(0, oi // 2)
                spill_x1f(1)
                d_rowsbc(1)
                for oi in range(OT):
                    c_step(2, oi)
                    if oi % 2 == 1:
                        d_apply_pair(1, oi // 2)
                spill_x1f(2)
                d_rowsbc(2)
                for kp in range(KT // 2):
                    d_apply_pair(2, kp)
                ost.close()

            # ---------- FFN single-pass ----------
            psg.close()
            # pool creation order fixes SBUF address reuse: wkp+wvcp (whose
            # first tiles are written late or wait harmlessly) land on the
            # o-proj scratch ranges still being read by the LN2 tail; sil
            # lands above them on long-dead TM space.  Otherwise the first
            # silu inherits a wait on the entire chunk-2 d_apply chain.
            with tc.tile_pool(name="mme", bufs=8, space="PSUM") as mme, \
                 tc.tile_pool(name="silup", bufs=1) as silup, \
                 tc.tile_pool(name="wkp", bufs=KG_HEAD) as wkp, \
                 tc.tile_pool(name="wvcp", bufs=3) as wvcp, \
                 tc.tile_pool(name="fsc", bufs=2) as fsc, \
                 tc.tile_pool(name="prp", bufs=2) as prp:
                sil = silup.tile([128, UPT, 1024], F8)
                wk_head = {}

                def load_wk(gi):
                    wk = wkp.tile([128, KG, KT, 128], F8, tag="wkg",
                                  name=f"wkg{gi}")
                    nc.sync.dma_start(wk[:], Wkey8[gi * KG : (gi + 1) * KG]
                                      .transpose([1, 0, 2, 3]))
                    return wk

                def key_chain(wk_g, gi, ui, cs, cw):
                    kps = mme.tile([128, 512], F32, tag="acc")
                    for ki in range(0, KT, 2):
                        nc.tensor.matmul(
                            kps[:, :cw], wk_g[:, ui - gi * KG, ki : ki + 2, :],
                            cmt[:, ki : ki + 2, cs : cs + cw],
                            start=(ki == 0), stop=(ki == KT - 2), perf_mode=DR)
                    nc.scalar.activation(sil[:, ui, cs : cs + cw],
                                         kps[:, :cw], AF.Silu, scale=ISW)

                # Pass A: columns 0..682 need only LN2 chunks 0/1 (done before
                # the FFN starts), so all of it runs while the chunk-2 LN2
                # apply tail finishes on DVE.  Pass B (columns 683..1023)
                # starts from the still-resident last groups (no DMA) and
                # re-streams the earlier groups.  Weight loads stay 1 group
                # ahead: cross-engine waits count ALL DMAs issued so far, so
                # a front-loaded burst would stall the silus.
                pending = load_wk(0)
                for gi in range(NKG):
                    wk_g = pending
                    wk_head[gi] = wk_g
                    if gi + 1 < NKG:
                        pending = load_wk(gi + 1)
                    for ui in range(gi * KG, (gi + 1) * KG):
                        key_chain(wk_g, gi, ui, 0, 512)
                    for ui in range(gi * KG, (gi + 1) * KG):
                        key_chain(wk_g, gi, ui, 512, 170)
                # last 5 groups still resident in wkp's bufs; consume them in
                # slot-rotation order so each reload can start as soon as its
                # slot's reader finishes
                for gi in range(NKG - 5, NKG):
                    for ui in range(gi * KG, (gi + 1) * KG):
                        key_chain(wk_head[gi], gi, ui, 682, 342)
                pending = load_wk(0)
                for gi in range(NKG - 5):
                    wk_g = pending
                    if gi + 1 < NKG - 5:
                        pending = load_wk(gi + 1)
                    for ui in range(gi * KG, (gi + 1) * KG):
                        key_chain(wk_g, gi, ui, 682, 342)

                for oi in range(OT):
                    wt = wvcp.tile([128, UPT + KT, 128], F8, tag="wvc")
                    nc.sync.dma_start(wt[:], Wvc[oi])
                    x1t = fsc.tile([128, 1024], F16, tag="x1r")
                    nc.sync.dma_start(x1t[:], x1f[oi, :, 2 : 2 + 1024])
                    prod = prp.tile([128, 1024], F32, tag="prod")
                    for hf in range(2):
                        cs = hf * 512
                        kvps = mme.tile([128, 512], F32, tag="acc")
                        for ki in range(0, UPT, 2):
                            nc.tensor.matmul(
                                kvps[:], wt[:, ki : ki + 2, :],
                                sil[:, ki : ki + 2, cs : cs + 512],
                                start=(ki == 0), stop=(ki == UPT - 2),
                                perf_mode=DR)
                        rrps = mme.tile([128, 512], F32, tag="acc")
                        for ki in range(0, KT, 2):
                            nc.tensor.matmul(
                                rrps[:], wt[:, UPT + ki : UPT + ki + 2, :],
                                cmt[:, ki : ki + 2, cs : cs + 512],
                                start=(ki == 0), stop=(ki == KT - 2),
                                perf_mode=DR)
                        sr = fsc.tile([128, 512], F16, tag="sr")
                        nc.scalar.activation(sr[:], rrps[:], AF.Sigmoid,
                                             scale=ISW)
                        nc.vector.scalar_tensor_tensor(
                            prod[:, cs : cs + 512], kvps[:], ISWV, sr[:],
                            op0=OP.mult, op1=OP.mult)
                        nc.vector.tensor_add(prod[:, cs : cs + 512],
                                             prod[:, cs : cs + 512],
                                             x1t[:, cs : cs + 512])
                    nc.sync.dma_start(out[oi], prod[:])
    nc.compile()
    return nc


def get_nc():
    if "nc" not in _BUILD_CACHE:
        _BUILD_CACHE["nc"] = build()
    return _BUILD_CACHE["nc"]


def make_in_maps(inputs):
    x = np.asarray(inputs["x"], dtype=np.float32)
    Wv8 = _p8(inputs["Wv"], SW)
    Wo8, Wolo = _p8_hilo(inputs["Wo"], SW)
    Wr8 = _p8(inputs["Wr"], SW)
    # combined panels: [OT, 128, parts, KT, 128]
    Wvr = np.ascontiguousarray(np.stack([Wv8, Wr8], axis=2))
    Woc = np.ascontiguousarray(np.stack([Wo8, Wolo], axis=2))
    Wval8 = _p8(inputs["Wval"], SWV)         # [OT, 128, UPT, 128]
    Wcr8 = _p8(inputs["Wcr"], SW)            # [OT, 128, KT, 128]
    Wvc = np.ascontiguousarray(np.concatenate([Wval8, Wcr8], axis=2))
    shared = {
        "Wvr": Wvr, "Woc": Woc,
        "Wkey8": np.ascontiguousarray(_p8(inputs["Wkey"], SW)),
        "Wvc": Wvc,
        "mixa": np.ascontiguousarray(np.stack(
            [_mix128(inputs["tm_mv"]), _mix128(inputs["tm_mr"]),
             _mix128(inputs["cm_mk"])], axis=1)),
    }
    in_maps = []
    for c in range(8):
        b, half = divmod(c, 2)
        s = half * 1024
        xs = np.zeros((TCORE, H), np.float32)
        lo = max(s - 2, 0)
        xs[2 - (s - lo):, :] = x[b, lo : s + 1024, :]
        xs16 = xs.T.astype(np.float16)                     # [H, TCORE]
        xp = xs16.reshape(KT, 128, TCORE).transpose(1, 0, 2)
        m = dict(shared)
        m["xT"] = np.ascontiguousarray(xp)
        xp8 = xp.astype(E4)
        m["xT8"] = np.ascontiguousarray(
            np.stack([xp8[:, :, 0:342], xp8[:, :, 342:684],
                      xp8[:, :, 684:1026]]))
        in_maps.append(m)
    return in_maps


def run(inputs, **kw):
    from concourse.bass_utils import run_bass_kernel_spmd

    in_maps = make_in_maps(inputs)
    nc = get_nc()
    res = run_bass_kernel_spmd(nc, in_maps, core_ids=list(range(8)), **kw)
    outa = np.empty((B, T, H), np.float32)
    for c in range(8):
        b, half = divmod(c, 2)
        o = res.results[c]["out"].reshape(H, 1024)
        outa[b, half * 1024 : (half + 1) * 1024, :] = o.T
    return outa, res


def kernel(**inputs):
    return run(inputs)[0]


# revision 63
# speedup vs baseline: 1.0531x; 1.0093x over previous
"""RWKV GPT block kernel for 8 Trainium2 NeuronCores — fp8 DoubleRow v5.

Quantization plan: v-proj 2-pass act-side hi/lo (vhi+vlo @ single fp8 Wv
panel), r/key/val/cr plain fp8 DoubleRow, o-proj 3-term hi/lo fp8, LN
stats via fp8 ones-matmuls.  (rel_err ~= 1.73e-2 vs the 2e-2 gate.)

Scheduling (vs the 520us v3 baseline):
- LN1 applies batched in ki-pairs (mul/add/sub on [128,2*hn] with
  stride-0 broadcast rows); vhi/vlo conversion batched per pair;
  o-epilogue ohi/olo conversions batched per oi-pair; LN2 applies
  batched the same way and kept entirely on DVE.  This fits the
  DVE/Act/Pool work under the PE pace in the mm_vr/c-step phases.
- Startup DMA order: chunk-0 x loads first, later chunks' x loads
  issued after the ops that must not wait on them (cross-engine waits
  snapshot full per-engine instruction counts at issue, so a DMA
  issued early can stall unrelated later consumers).
- FFN key matmuls in 4-ui groups, split by token columns: pass A
  (cols 0..681, needing only LN2 chunks 0/1) streams all 16 groups
  while the chunk-2 LN2 tail runs on DVE; pass B (cols 682..1023)
  starts from the 5 still-resident groups, then re-streams the rest.
- SBUF pool address layout chosen so the FFN tiles (sil, wkg, wvc)
  never land on ranges written by DVE/DMA late in the o-proj phase:
  sil sits over the Act/Pool-written ohi/olo/vhi/vlo block, and the
  d_apply scratch + x2b sit above everything the FFN pools reach.
  (Misplacing sil costs ~15us: its first write inherits a wait on the
  whole LN2 tail via the conservative count-based semaphores.)
"""
import sys

sys.path.insert(0, "/opt/trn_rl_repo")
sys.path.insert(0, "/opt/pypackages")

import numpy as np
import ml_dtypes

H = 2048
KT = H // 128
OT = H // 128
UPT = 4 * H // 128
B = 4
T = 2048
TCORE = 1026
EPS = 1e-5
INV_H = 1.0 / H
SW = 32.0
SWV = 64.0
ISW = 1.0 / SW
ISWV = 1.0 / SWV
E4 = ml_dtypes.float8_e4m3

CH = [(1, 342), (342, 684), (684, 1026)]
ST1 = [(0, 342), (342, 684), (684, 1026)]
KG = 4                 # FFN key-group size (uis per weight hold)
NKG = UPT // KG
KG_HEAD = 5            # groups whose half-0 chains run before any half-1

_BUILD_CACHE = {}


def _panels(WT):
    IN, OUT = WT.shape
    kt, ot = IN // 128, OUT // 128
    return np.ascontiguousarray(
        WT.reshape(kt, 128, ot, 128).transpose(2, 1, 0, 3))


def _p8(W, scale):
    a = _panels(np.asarray(W, np.float32).T) * scale
    return a.astype(E4)


def _p8_hilo(W, scale):
    a = _panels(np.asarray(W, np.float32).T) * scale
    hi = a.astype(E4)
    lo = (a - hi.astype(np.float32)).astype(E4)
    return hi, lo


def _mix128(v):
    return np.ascontiguousarray(
        np.asarray(v, dtype=np.float32).reshape(-1)[:H].reshape(KT, 128).T)


def build():
    import contextlib

    import concourse.bacc as bacc
    import concourse.mybir as mybir
    import concourse.tile as tile

    F16 = mybir.dt.float16
    F32 = mybir.dt.float32
    F8 = mybir.dt.float8e4
    AF = mybir.ActivationFunctionType
    OP = mybir.AluOpType
    DR = mybir.MatmulPerfMode.DoubleRow

    nc = bacc.Bacc("TRN2", target_bir_lowering=False)

    xT = nc.dram_tensor("xT", [128, KT, TCORE], F16, kind="ExternalInput")
    xT8 = nc.dram_tensor("xT8", [3, 128, KT, 342], F8, kind="ExternalInput")
    Wvr = nc.dram_tensor("Wvr", [OT, 128, 2, KT, 128], F8, kind="ExternalInput")
    Woc = nc.dram_tensor("Woc", [OT, 128, 2, KT, 128], F8, kind="ExternalInput")
    Wkey8 = nc.dram_tensor("Wkey8", [UPT, 128, KT, 128], F8, kind="ExternalInput")
    Wvc = nc.dram_tensor("Wvc", [OT, 128, UPT + KT, 128], F8,
                         kind="ExternalInput")
    mixa = nc.dram_tensor("mixa", [128, 3, KT], F32, kind="ExternalInput")
    out = nc.dram_tensor("out", [OT, 128, 1024], F32, kind="ExternalOutput")
    x1f = nc.dram_tensor("x1f", [OT, 128, TCORE], F16, kind="Internal")

    with tile.TileContext(nc) as tc, contextlib.ExitStack() as g:
        cpool = g.enter_context(tc.tile_pool(name="consts", bufs=1))
        psg = contextlib.ExitStack()
        st = psg.enter_context(tc.tile_pool(name="st", bufs=1, space="PSUM"))
        mm = psg.enter_context(tc.tile_pool(name="mm", bufs=6, space="PSUM"))
        rows = g.enter_context(tc.tile_pool(name="rows", bufs=2))
        rsc = g.enter_context(tc.tile_pool(name="rsc", bufs=1))
        bcs = g.enter_context(tc.tile_pool(name="bcs", bufs=2))
        sqp = g.enter_context(tc.tile_pool(name="sqp", bufs=1))
        ones_r = cpool.tile([1, 128], F16)
        nc.vector.memset(ones_r[:], 1.0)
        ones8 = cpool.tile([128, 2, 128], F8)
        nc.vector.memset(ones8[:], 1.0)
        mixt = cpool.tile([128, 3, KT], F32)
        nc.sync.dma_start(mixt[:], mixa[:])
        mv = mixt[:, 0]
        mr = mixt[:, 1]
        mk = mixt[:, 2]

        def stats_rows(src8, n, sq_pre=None, split_sq=False):
            s1 = st.tile([128, 512], F32, tag="s1")
            s2 = st.tile([128, 512], F32, tag="s2")
            if sq_pre is None:
                sq = sqp.tile([128, KT, 342], F8, tag="sq")
                for ki in range(KT):
                    # split_sq alternates the squares between Act and DVE:
                    # at kernel start the serial 16-square Act chain gates
                    # LN1 stats -> bcast -> applies -> everything
                    if split_sq and ki % 2 == 1:
                        nc.vector.tensor_mul(sq[:, ki, :n], src8[:, ki, :],
                                             src8[:, ki, :])
                    else:
                        nc.scalar.square(sq[:, ki, :n], src8[:, ki, :])
            else:
                sq = sq_pre
            for ki in range(0, KT, 2):
                nc.tensor.matmul(s1[:, :n], ones8[:], src8[:, ki : ki + 2, :],
                                 start=(ki == 0), stop=(ki == KT - 2),
                                 perf_mode=DR)
            for ki in range(0, KT, 2):
                nc.tensor.matmul(s2[:, :n], ones8[:], sq[:, ki : ki + 2, :n],
                                 start=(ki == 0), stop=(ki == KT - 2),
                                 perf_mode=DR)
            m = rsc.tile([1, 512], F32, tag="m")
            nc.vector.tensor_scalar_mul(m[:, :n], s1[0:1, :n], INV_H)
            var = rsc.tile([1, 512], F32, tag="var")
            nc.vector.tensor_scalar_mul(var[:, :n], s2[0:1, :n], INV_H)
            msq = rsc.tile([1, 512], F32, tag="msd")
            nc.vector.tensor_mul(msq[:, :n], m[:, :n], m[:, :n])
            nc.vector.tensor_sub(var[:, :n], var[:, :n], msq[:, :n])
            nc.vector.tensor_scalar_add(var[:, :n], var[:, :n], EPS)
            sd = rsc.tile([1, 512], F32, tag="msd")
            nc.scalar.sqrt(sd[:, :n], var[:, :n])
            a_rf = rsc.tile([1, 512], F32, tag="var")
            nc.vector.reciprocal(a_rf[:, :n], sd[:, :n])
            a_rc = rows.tile([1, 512], F16, tag="arow")
            nc.vector.tensor_copy(a_rc[:, :n], a_rf[:, :n])
            c_rc = rows.tile([1, 512], F16, tag="crow")
            nc.vector.scalar_tensor_tensor(
                c_rc[:, :n], m[:, :n], -1.0, a_rf[:, :n],
                op0=OP.mult, op1=OP.mult)
            return a_rc, c_rc

        def bcast2(a_rc, c_rc, n, prev, hn):
            off = hn - n
            abp = st.tile([128, 512], F32, tag="s1")
            cbp = st.tile([128, 512], F32, tag="s2")
            if off:
                pa_rc, pc_rc, pn = prev
                nc.tensor.matmul(abp[:, 0:1], ones_r[:], pa_rc[:, pn - 1 : pn],
                                 start=True, stop=True, skip_group_check=True)
                nc.tensor.matmul(cbp[:, 0:1], ones_r[:], pc_rc[:, pn - 1 : pn],
                                 start=True, stop=True, skip_group_check=True)
            nc.tensor.matmul(abp[:, off : off + n], ones_r[:], a_rc[:, :n],
                             start=True, stop=True, skip_group_check=True)
            nc.tensor.matmul(cbp[:, off : off + n], ones_r[:], c_rc[:, :n],
                             start=True, stop=True, skip_group_check=True)
            ab = bcs.tile([128, 512], F16, tag="ab")
            nc.scalar.copy(ab[:, :hn], abp[:, :hn])
            cb = bcs.tile([128, 512], F16, tag="cb")
            nc.scalar.copy(cb[:, :hn], cbp[:, :hn])
            return ab, cb

        with tc.tile_pool(name="cmp", bufs=1) as cmp_:
            cmt = cmp_.tile([128, KT, 1024], F8)

            # ohi/olo/vhi/vlo are Act/Pool-written only: keeping them first
            # gives the FFN sil tile a 64KB window free of DVE/DMA-written
            # space (cross-engine waits snapshot full engine counts, so any
            # DVE/DMA overlap would stall the silus behind the LN2 tail).
            with tc.tile_pool(name="ohip", bufs=1) as ohip, \
                 tc.tile_pool(name="olop", bufs=1) as olop, \
                 tc.tile_pool(name="vhip", bufs=1) as vhip, \
                 tc.tile_pool(name="vlop", bufs=1) as vlop, \
                 tc.tile_pool(name="xtlp", bufs=1) as xtlp:
                ohi = [ohip.tile([128, KT, pb - pa], F8, tag=f"ohi{ci}",
                                 name=f"ohi{ci}")
                       for ci, (pa, pb) in enumerate(CH)]
                olo = [olop.tile([128, KT, pb - pa], F8, tag=f"olo{ci}",
                                 name=f"olo{ci}")
                       for ci, (pa, pb) in enumerate(CH)]
                vhi = [vhip.tile([128, KT, pb - pa], F8, tag=f"vhi{ci}",
                                 name=f"vhi{ci}")
                       for ci, (pa, pb) in enumerate(CH)]
                vlo = [vlop.tile([128, KT, pb - pa], F8, tag=f"vlo{ci}",
                                 name=f"vlo{ci}")
                       for ci, (pa, pb) in enumerate(CH)]
                xtl = []
                for ci, (sa, sb) in enumerate(ST1):
                    ha = max(sa - 1, 0)
                    xtl.append(xtlp.tile([128, KT, sb - ha], F16,
                                         tag=f"xtl{ci}", name=f"xtl{ci}"))

                with tc.tile_pool(name="x8p", bufs=1) as x8p, \
                     tc.tile_pool(name="rinp", bufs=1) as rinp, \
                     tc.tile_pool(name="p1sc", bufs=2) as p1sc, \
                     tc.tile_pool(name="hp", bufs=2) as hp, \
                     tc.tile_pool(name="wvrp", bufs=5) as wvrp, \
                     tc.tile_pool(name="sgp", bufs=3) as sgp, \
                     tc.tile_pool(name="vsbp", bufs=3) as vsbp, \
                     tc.tile_pool(name="oisc", bufs=2) as oisc:
                    rin = [rinp.tile([128, KT, pb - pa], F8, tag=f"rin{ci}",
                                     name=f"rin{ci}")
                           for ci, (pa, pb) in enumerate(CH)]
                    x8l = [x8p.tile([128, KT, 342], F8, tag=f"x8l{ci}",
                                    name=f"x8l{ci}") for ci in range(3)]

                    def load_x8(ci):
                        nc.sync.dma_start(x8l[ci][:], xT8[ci])

                    def load_xtl(ci):
                        sa, sb = ST1[ci]
                        ha = max(sa - 1, 0)
                        nc.sync.dma_start(xtl[ci][:], xT[:, :, ha:sb])

                    load_x8(0)
                    load_xtl(0)
                    ln1 = {}

                    def ln1_rowsbc(ci, prev):
                        sa, sb = ST1[ci]
                        ha = max(sa - 1, 0)
                        hn = sb - ha
                        n = sb - sa
                        a_rc, c_rc = stats_rows(x8l[ci][:, :, :n], n,
                                                split_sq=(ci == 0))
                        ln1[ci] = (bcast2(a_rc, c_rc, n, prev, hn), ha, hn)
                        return (a_rc, c_rc, n)

                    def ln1_apply_pair(ci, kp, pool_mul=False):
                        """Batched LN1 apply + mix for ki = 2kp, 2kp+1.

                        (pool_mul is unused: routing the rin STTs or the
                        mul to GpSimd fails neuronxcc's engine check for
                        TensorScalarPtr / was Pool-pace-limited.)
                        """
                        (ab, cb), ha, hn = ln1[ci]
                        nmix = hn - 1
                        k0 = 2 * kp
                        ab_b = ab[:, :hn].unsqueeze(1).broadcast_to(
                            [128, 2, hn])
                        cb_b = cb[:, :hn].unsqueeze(1).broadcast_to(
                            [128, 2, hn])
                        tt = p1sc.tile([128, 2, hn], F16, tag="tt")
                        nc.vector.tensor_mul(tt[:], xtl[ci][:, k0 : k0 + 2, :],
                                             ab_b)
                        h = hp.tile([128, 2, hn], F16, tag="h")
                        nc.vector.tensor_add(h[:], tt[:], cb_b)
                        d = p1sc.tile([128, 2, hn], F16, tag="d")
                        hf = h[:].rearrange("p a b -> p (a b)")
                        df = d[:].rearrange("p a b -> p (a b)")
                        nc.vector.tensor_sub(df[:, : 2 * hn - 1],
                                             hf[:, 1 : 2 * hn],
                                             hf[:, : 2 * hn - 1])
                        v16 = p1sc.tile([128, 2, nmix], F16, tag="v16")
                        for j in range(2):
                            ki = k0 + j
                            nc.vector.scalar_tensor_tensor(
                                v16[:, j, :], d[:, j, :nmix],
                                mv[:, ki : ki + 1], h[:, j, :nmix],
                                op0=OP.mult, op1=OP.add)
                            nc.vector.scalar_tensor_tensor(
                                rin[ci][:, ki, :], d[:, j, :nmix],
                                mr[:, ki : ki + 1], h[:, j, :nmix],
                                op0=OP.mult, op1=OP.add)
                        nc.scalar.copy(vhi[ci][:, k0 : k0 + 2, :], v16[:])
                        nc.gpsimd.tensor_sub(vlo[ci][:, k0 : k0 + 2, :],
                                             v16[:], vhi[ci][:, k0 : k0 + 2, :])

                    def mm_vr(ci, extra=None):
                        pa, pb = CH[ci]
                        n = pb - pa
                        oi16 = {}
                        for oi in range(OT):
                            wt = wvrp.tile([128, 2, KT, 128], F8, tag="wvr")
                            nc.sync.dma_start(wt[:], Wvr[oi])
                            vps = mm.tile([128, 512], F32, tag="acc")
                            for ki in range(0, KT, 2):
                                nc.tensor.matmul(
                                    vps[:, :n], wt[:, 0, ki : ki + 2, :],
                                    vhi[ci][:, ki : ki + 2, :],
                                    start=(ki == 0), stop=False, perf_mode=DR)
                            for ki in range(0, KT, 2):
                                nc.tensor.matmul(
                                    vps[:, :n], wt[:, 0, ki : ki + 2, :],
                                    vlo[ci][:, ki : ki + 2, :],
                                    start=False, stop=(ki == KT - 2),
                                    perf_mode=DR)
                            rps = mm.tile([128, 512], F32, tag="acc")
                            for ki in range(0, KT, 2):
                                nc.tensor.matmul(
                                    rps[:, :n], wt[:, 1, ki : ki + 2, :],
                                    rin[ci][:, ki : ki + 2, :],
                                    start=(ki == 0), stop=(ki == KT - 2),
                                    perf_mode=DR)
                            # interleaved work issued AFTER this oi's chains:
                            # count-based semaphores make a chain wait on any
                            # DVE work issued before it, so issuing the next
                            # chunk's apply first would serialize PE behind it
                            if extra is not None:
                                extra(oi)
                            sg = sgp.tile([128, 512], F16, tag="sg")
                            nc.scalar.activation(sg[:, :n], rps[:, :n],
                                                 AF.Sigmoid, scale=ISW)
                            vsb = vsbp.tile([128, 512], F16, tag="vsb")
                            nc.scalar.activation(vsb[:, :n], vps[:, :n],
                                                 AF.Copy, scale=ISW)
                            if oi % 2 == 0:
                                oi16[oi // 2] = oisc.tile(
                                    [128, 2, n], F16, tag="oi16",
                                    name=f"oi16_{ci}_{oi}")
                            cur = oi16[oi // 2]
                            nc.vector.tensor_mul(cur[:, oi % 2, :],
                                                 sg[:, :n], vsb[:, :n])
                            if oi % 2 == 1:
                                o0 = oi - 1
                                nc.scalar.copy(ohi[ci][:, o0 : o0 + 2, :],
                                               cur[:])
                                nc.gpsimd.tensor_sub(
                                    olo[ci][:, o0 : o0 + 2, :], cur[:],
                                    ohi[ci][:, o0 : o0 + 2, :])

                    pr = ln1_rowsbc(0, None)
                    for kp in range(KT // 2):
                        ln1_apply_pair(0, kp, pool_mul=True)
                    load_x8(1)
                    pr1 = ln1_rowsbc(1, pr)
                    load_xtl(1)

                    def extra_c1(oi):
                        if oi == 8:
                            load_x8(2)
                        if oi == 12:
                            load_xtl(2)
                        if oi % 2 == 1:
                            ln1_apply_pair(1, oi // 2)

                    mm_vr(0, extra=extra_c1)
                    ln1_rowsbc(2, pr1)
                    mm_vr(1, extra=lambda oi: (
                        ln1_apply_pair(2, oi // 2) if oi % 2 == 1 else None))
                    mm_vr(2)

                # ---------- o-proj + residual + LN2 ----------
                # pool order fixes SBUF address reuse for the FFN pools
                # stacked after these pop: the d_apply scratch (dsc/h2p/cyp,
                # DVE-written until the chunk-2 LN2 tail ends) must land
                # under wvcp (first written in the val phase), never under
                # silup — else the first silu inherits an ~18us wait.
                # address order (low→high): x2b8 (Act-written) and wop (DMA)
                # land under the FFN's wkp/wvcp claims; x2b and the d_apply
                # scratch sit above everything the FFN pools reach, so no FFN
                # write inherits a wait on the LN2 tail or the spills.
                ost = contextlib.ExitStack()
                x2b8p = ost.enter_context(tc.tile_pool(name="x2b8", bufs=2))
                wop = ost.enter_context(tc.tile_pool(name="wop", bufs=4))
                x2bp = ost.enter_context(tc.tile_pool(name="x2b", bufs=1))
                dsc = ost.enter_context(tc.tile_pool(name="dsc", bufs=3))
                h2p = ost.enter_context(tc.tile_pool(name="h2p", bufs=2))
                cyp = ost.enter_context(tc.tile_pool(name="cyp", bufs=1))
                x2b = {}
                x2b8 = {}
                ln2 = {}
                carry = {}
                for ci in range(3):
                    x2b[ci] = x2bp.tile([128, KT, 342], F16,
                                        tag=f"x2b{ci}", name=f"x2b{ci}")
                    carry[ci] = cyp.tile([128, KT], F16, tag=f"cy{ci}",
                                         name=f"cy{ci}")

                def c_step(ci, oi):
                    pa, pb = CH[ci]
                    sa, sb = ST1[ci]
                    ha = max(sa - 1, 0)
                    n = pb - pa
                    if oi == 0:
                        x2b8[ci] = x2b8p.tile([128, KT, 342], F8,
                                              tag="x2b8", name=f"x2b8{ci}")
                    wt = wop.tile([128, 2, KT, 128], F8, tag="wo")
                    nc.sync.dma_start(wt[:], Woc[oi])
                    ops_ = mm.tile([128, 512], F32, tag="acc")
                    for ki in range(0, KT, 2):
                        nc.tensor.matmul(
                            ops_[:, :n], wt[:, 0, ki : ki + 2, :],
                            ohi[ci][:, ki : ki + 2, :],
                            start=(ki == 0), stop=False, perf_mode=DR)
                    for ki in range(0, KT, 2):
                        nc.tensor.matmul(
                            ops_[:, :n], wt[:, 1, ki : ki + 2, :],
                            ohi[ci][:, ki : ki + 2, :],
                            start=False, stop=False, perf_mode=DR)
                    for ki in range(0, KT, 2):
                        nc.tensor.matmul(
                            ops_[:, :n], wt[:, 0, ki : ki + 2, :],
                            olo[ci][:, ki : ki + 2, :],
                            start=False, stop=(ki == KT - 2), perf_mode=DR)
                    nc.vector.scalar_tensor_tensor(
                        x2b[ci][:, oi, :n], ops_[:, :n], ISW,
                        xtl[ci][:, oi, pa - ha : pb - ha],
                        op0=OP.mult, op1=OP.add)
                    nc.scalar.copy(x2b8[ci][:, oi, :n], x2b[ci][:, oi, :n])

                def d_rowsbc(ci, sq_pre=None):
                    pa, pb = CH[ci]
                    n = pb - pa
                    a_rc, c_rc = stats_rows(x2b8[ci][:, :, :n], n,
                                            sq_pre=sq_pre)
                    ln2[ci] = bcast2(a_rc, c_rc, n, None, n)

                def d_apply_pair(ci, kp):
                    """Batched LN2 apply + mix for ki = 2kp, 2kp+1.

                    Chunks 0/1 put the x*ab multiply on GpSimd (idle during
                    the c-phases).  Chunk 2 stays entirely on DVE: its tail
                    runs concurrently with the first FFN silus, and any Pool
                    work issued there would stall them via the sil tile's
                    count-based Pool anti-dependency (sil sits over the
                    Pool-written olo/vlo ranges).
                    """
                    pa, pb = CH[ci]
                    n = pb - pa
                    k0 = 2 * kp
                    ab, cb = ln2[ci]
                    ab_b = ab[:, :n].unsqueeze(1).broadcast_to([128, 2, n])
                    cb_b = cb[:, :n].unsqueeze(1).broadcast_to([128, 2, n])
                    tt = dsc.tile([128, 2, n], F16, tag="tt2")
                    nc.vector.tensor_mul(tt[:], x2b[ci][:, k0 : k0 + 2, :n],
                                         ab_b)
                    h2 = h2p.tile([128, 2, n], F16, tag="h2")
                    nc.vector.tensor_add(h2[:], tt[:], cb_b)
                    if ci > 0:
                        pcy = carry[ci - 1]
                        db = dsc.tile([128, 2], F16, tag="db")
                        nc.vector.tensor_sub(
                            db[:], h2[:, :, 0], pcy[:, k0 : k0 + 2])
                        gidx = pa - 2
                        for j in range(2):
                            ki = k0 + j
                            nc.vector.scalar_tensor_tensor(
                                cmt[:, ki, gidx : gidx + 1], db[:, j : j + 1],
                                mk[:, ki : ki + 1], pcy[:, ki : ki + 1],
                                op0=OP.mult, op1=OP.add)
                    d2 = dsc.tile([128, 2, n], F16, tag="d2")
                    h2f = h2[:].rearrange("p a b -> p (a b)")
                    d2f = d2[:].rearrange("p a b -> p (a b)")
                    nc.vector.tensor_sub(d2f[:, : 2 * n - 1],
                                         h2f[:, 1 : 2 * n],
                                         h2f[:, : 2 * n - 1])
                    glo, ghi = pa - 1, pb - 2
                    for j in range(2):
                        ki = k0 + j
                        nc.vector.scalar_tensor_tensor(
                            cmt[:, ki, glo:ghi], d2[:, j, : ghi - glo],
                            mk[:, ki : ki + 1], h2[:, j, : ghi - glo],
                            op0=OP.mult, op1=OP.add)
                    nc.vector.tensor_copy(carry[ci][:, k0 : k0 + 2],
                                          h2[:, :, n - 1])

                def spill_x1f(ci):
                    pa, pb = CH[ci]
                    n = pb - pa
                    nc.sync.dma_start(
                        x1f[:, :, pa:pb].transpose([1, 0, 2]),
                        x2b[ci][:, :, :n])

                for oi in range(OT):
                    c_step(0, oi)
                spill_x1f(0)
                d_rowsbc(0)
                for oi in range(OT):
                    c_step(1, oi)
                    if oi % 2 == 1:
                        d_apply_pair(0, oi // 2)
                spill_x1f(1)
                d_rowsbc(1)
                # chunk-2 stats squares computed per-oi inside the loop so
                # d_rowsbc(2) doesn't serialize a 16-square Act chain at the
                # FFN boundary
                sq2 = sqp.tile([128, KT, 342], F8, tag="sq")
                n2 = CH[2][1] - CH[2][0]
                for oi in range(OT):
                    c_step(2, oi)
                    nc.scalar.square(sq2[:, oi, :n2], x2b8[2][:, oi, :n2])
                    if oi % 2 == 1:
                        d_apply_pair(1, oi // 2)
                spill_x1f(2)
                d_rowsbc(2, sq_pre=sq2)
                for kp in range(KT // 2):
                    d_apply_pair(2, kp)
                ost.close()

            # ---------- FFN single-pass ----------
            psg.close()
            # pool creation order fixes SBUF address reuse: wkp+wvcp (whose
            # first tiles are written late or wait harmlessly) land on the
            # o-proj scratch ranges still being read by the LN2 tail; sil
            # lands above them on long-dead TM space.  Otherwise the first
            # silu inherits a wait on the entire chunk-2 d_apply chain.
            with tc.tile_pool(name="mme", bufs=8, space="PSUM") as mme, \
                 tc.tile_pool(name="silup", bufs=1) as silup, \
                 tc.tile_pool(name="wkp", bufs=KG_HEAD) as wkp, \
                 tc.tile_pool(name="wvcp", bufs=3) as wvcp, \
                 tc.tile_pool(name="fsc", bufs=2) as fsc, \
                 tc.tile_pool(name="prp", bufs=2) as prp:
                sil = silup.tile([128, UPT, 1024], F8)
                wk_head = {}

                def load_wk(gi):
                    wk = wkp.tile([128, KG, KT, 128], F8, tag="wkg",
                                  name=f"wkg{gi}")
                    nc.sync.dma_start(wk[:], Wkey8[gi * KG : (gi + 1) * KG]
                                      .transpose([1, 0, 2, 3]))
                    return wk

                def key_chain(wk_g, gi, ui, cs, cw):
                    kps = mme.tile([128, 512], F32, tag="acc")
                    for ki in range(0, KT, 2):
                        nc.tensor.matmul(
                            kps[:, :cw], wk_g[:, ui - gi * KG, ki : ki + 2, :],
                            cmt[:, ki : ki + 2, cs : cs + cw],
                            start=(ki == 0), stop=(ki == KT - 2), perf_mode=DR)
                    nc.scalar.activation(sil[:, ui, cs : cs + cw],
                                         kps[:, :cw], AF.Silu, scale=ISW)

                # Pass A: columns 0..682 need only LN2 chunks 0/1 (done before
                # the FFN starts), so all of it runs while the chunk-2 LN2
                # apply tail finishes on DVE.  Pass B (columns 683..1023)
                # starts from the still-resident last groups (no DMA) and
                # re-streams the earlier groups.  Weight loads stay 1 group
                # ahead: cross-engine waits count ALL DMAs issued so far, so
                # a front-loaded burst would stall the silus.
                pending = load_wk(0)
                for gi in range(NKG):
                    wk_g = pending
                    wk_head[gi] = wk_g
                    if gi + 1 < NKG:
                        pending = load_wk(gi + 1)
                    for ui in range(gi * KG, (gi + 1) * KG):
                        key_chain(wk_g, gi, ui, 0, 512)
                    for ui in range(gi * KG, (gi + 1) * KG):
                        key_chain(wk_g, gi, ui, 512, 170)
                # last 5 groups still resident in wkp's bufs; consume them in
                # slot-rotation order so each reload can start as soon as its
                # slot's reader finishes
                for gi in range(NKG - 5, NKG):
                    for ui in range(gi * KG, (gi + 1) * KG):
                        key_chain(wk_head[gi], gi, ui, 682, 342)
                pending = load_wk(0)
                for gi in range(NKG - 5):
                    wk_g = pending
                    if gi + 1 < NKG - 5:
                        pending = load_wk(gi + 1)
                    for ui in range(gi * KG, (gi + 1) * KG):
                        key_chain(wk_g, gi, ui, 682, 342)

                for oi in range(OT):
                    wt = wvcp.tile([128, UPT + KT, 128], F8, tag="wvc")
                    nc.sync.dma_start(wt[:], Wvc[oi])
                    x1t = fsc.tile([128, 1024], F16, tag="x1r")
                    nc.sync.dma_start(x1t[:], x1f[oi, :, 2 : 2 + 1024])
                    prod = prp.tile([128, 1024], F32, tag="prod")
                    for hf in range(2):
                        cs = hf * 512
                        kvps = mme.tile([128, 512], F32, tag="acc")
                        for ki in range(0, UPT, 2):
                            nc.tensor.matmul(
                                kvps[:], wt[:, ki : ki + 2, :],
                                sil[:, ki : ki + 2, cs : cs + 512],
                                start=(ki == 0), stop=(ki == UPT - 2),
                                perf_mode=DR)
                        rrps = mme.tile([128, 512], F32, tag="acc")
                        for ki in range(0, KT, 2):
                            nc.tensor.matmul(
                                rrps[:], wt[:, UPT + ki : UPT + ki + 2, :],
                                cmt[:, ki : ki + 2, cs : cs + 512],
                                start=(ki == 0), stop=(ki == KT - 2),
                                perf_mode=DR)
                        sr = fsc.tile([128, 512], F16, tag="sr")
                        nc.scalar.activation(sr[:], rrps[:], AF.Sigmoid,
                                             scale=ISW)
                        nc.vector.scalar_tensor_tensor(
                            prod[:, cs : cs + 512], kvps[:], ISWV, sr[:],
                            op0=OP.mult, op1=OP.mult)
                        nc.vector.tensor_add(prod[:, cs : cs + 512],
                                             prod[:, cs : cs + 512],
                                             x1t[:, cs : cs + 512])
                        # per-half out spill: the half-0 store overlaps the
                        # half-1 chains, shortening the end-of-kernel drain
                        nc.sync.dma_start(out[oi][:, cs : cs + 512],
                                          prod[:, cs : cs + 512])
    nc.compile()
    return nc


def get_nc():
    if "nc" not in _BUILD_CACHE:
        _BUILD_CACHE["nc"] = build()
    return _BUILD_CACHE["nc"]


def make_in_maps(inputs):
    x = np.asarray(inputs["x"], dtype=np.float32)
    Wv8 = _p8(inputs["Wv"], SW)
    Wo8, Wolo = _p8_hilo(inputs["Wo"], SW)
    Wr8 = _p8(inputs["Wr"], SW)
    # combined panels: [OT, 128, parts, KT, 128]
    Wvr = np.ascontiguousarray(np.stack([Wv8, Wr8], axis=2))
    Woc = np.ascontiguousarray(np.stack([Wo8, Wolo], axis=2))
    Wval8 = _p8(inputs["Wval"], SWV)         # [OT, 128, UPT, 128]
    Wcr8 = _p8(inputs["Wcr"], SW)            # [OT, 128, KT, 128]
    Wvc = np.ascontiguousarray(np.concatenate([Wval8, Wcr8], axis=2))
    shared = {
        "Wvr": Wvr, "Woc": Woc,
        "Wkey8": np.ascontiguousarray(_p8(inputs["Wkey"], SW)),
        "Wvc": Wvc,
        "mixa": np.ascontiguousarray(np.stack(
            [_mix128(inputs["tm_mv"]), _mix128(inputs["tm_mr"]),
             _mix128(inputs["cm_mk"])], axis=1)),
    }
    in_maps = []
    for c in range(8):
        b, half = divmod(c, 2)
        s = half * 1024
        xs = np.zeros((TCORE, H), np.float32)
        lo = max(s - 2, 0)
        xs[2 - (s - lo):, :] = x[b, lo : s + 1024, :]
        xs16 = xs.T.astype(np.float16)                     # [H, TCORE]
        xp = xs16.reshape(KT, 128, TCORE).transpose(1, 0, 2)
        m = dict(shared)
        m["xT"] = np.ascontiguousarray(xp)
        xp8 = xp.astype(E4)
        m["xT8"] = np.ascontiguousarray(
            np.stack([xp8[:, :, 0:342], xp8[:, :, 342:684],
                      xp8[:, :, 684:1026]]))
        in_maps.append(m)
    return in_maps


def run(inputs, **kw):
    from concourse.bass_utils import run_bass_kernel_spmd

    in_maps = make_in_maps(inputs)
    nc = get_nc()
    res = run_bass_kernel_spmd(nc, in_maps, core_ids=list(range(8)), **kw)
    outa = np.empty((B, T, H), np.float32)
    for c in range(8):
        b, half = divmod(c, 2)
        o = res.results[c]["out"].reshape(H, 1024)
        outa[b, half * 1024 : (half + 1) * 1024, :] = o.T
    return outa, res


def kernel(**inputs):
    return run(inputs)[0]


# revision 64
# speedup vs baseline: 1.0537x; 1.0006x over previous
"""RWKV GPT block kernel for 8 Trainium2 NeuronCores — fp8 DoubleRow v5.

Quantization plan: v-proj 2-pass act-side hi/lo (vhi+vlo @ single fp8 Wv
panel), r/key/val/cr plain fp8 DoubleRow, o-proj 3-term hi/lo fp8, LN
stats via fp8 ones-matmuls.  (rel_err ~= 1.73e-2 vs the 2e-2 gate.)

Scheduling (vs the 520us v3 baseline):
- LN1 applies batched in ki-pairs (mul/add/sub on [128,2*hn] with
  stride-0 broadcast rows); vhi/vlo conversion batched per pair;
  o-epilogue ohi/olo conversions batched per oi-pair; LN2 applies
  batched the same way and kept entirely on DVE.  This fits the
  DVE/Act/Pool work under the PE pace in the mm_vr/c-step phases.
- Startup DMA order: chunk-0 x loads first, later chunks' x loads
  issued after the ops that must not wait on them (cross-engine waits
  snapshot full per-engine instruction counts at issue, so a DMA
  issued early can stall unrelated later consumers).
- FFN key matmuls in 4-ui groups, split by token columns: pass A
  (cols 0..681, needing only LN2 chunks 0/1) streams all 16 groups
  while the chunk-2 LN2 tail runs on DVE; pass B (cols 682..1023)
  starts from the 5 still-resident groups, then re-streams the rest.
- SBUF pool address layout chosen so the FFN tiles (sil, wkg, wvc)
  never land on ranges written by DVE/DMA late in the o-proj phase:
  sil sits over the Act/Pool-written ohi/olo/vhi/vlo block, and the
  d_apply scratch + x2b sit above everything the FFN pools reach.
  (Misplacing sil costs ~15us: its first write inherits a wait on the
  whole LN2 tail via the conservative count-based semaphores.)
"""
import sys

sys.path.insert(0, "/opt/trn_rl_repo")
sys.path.insert(0, "/opt/pypackages")

import numpy as np
import ml_dtypes

H = 2048
KT = H // 128
OT = H // 128
UPT = 4 * H // 128
B = 4
T = 2048
TCORE = 1026
EPS = 1e-5
INV_H = 1.0 / H
SW = 32.0
SWV = 64.0
ISW = 1.0 / SW
ISWV = 1.0 / SWV
E4 = ml_dtypes.float8_e4m3

CH = [(1, 342), (342, 684), (684, 1026)]
ST1 = [(0, 342), (342, 684), (684, 1026)]
KG = 4                 # FFN key-group size (uis per weight hold)
NKG = UPT // KG
KG_HEAD = 5            # groups whose half-0 chains run before any half-1

_BUILD_CACHE = {}


def _panels(WT):
    IN, OUT = WT.shape
    kt, ot = IN // 128, OUT // 128
    return np.ascontiguousarray(
        WT.reshape(kt, 128, ot, 128).transpose(2, 1, 0, 3))


def _p8(W, scale):
    a = _panels(np.asarray(W, np.float32).T) * scale
    return a.astype(E4)


def _p8_hilo(W, scale):
    a = _panels(np.asarray(W, np.float32).T) * scale
    hi = a.astype(E4)
    lo = (a - hi.astype(np.float32)).astype(E4)
    return hi, lo


def _mix128(v):
    return np.ascontiguousarray(
        np.asarray(v, dtype=np.float32).reshape(-1)[:H].reshape(KT, 128).T)


def build():
    import contextlib

    import concourse.bacc as bacc
    import concourse.mybir as mybir
    import concourse.tile as tile

    F16 = mybir.dt.float16
    F32 = mybir.dt.float32
    F8 = mybir.dt.float8e4
    AF = mybir.ActivationFunctionType
    OP = mybir.AluOpType
    DR = mybir.MatmulPerfMode.DoubleRow

    nc = bacc.Bacc("TRN2", target_bir_lowering=False)

    xT = nc.dram_tensor("xT", [128, KT, TCORE], F16, kind="ExternalInput")
    xT8 = nc.dram_tensor("xT8", [3, 128, KT, 342], F8, kind="ExternalInput")
    Wvr = nc.dram_tensor("Wvr", [OT, 128, 2, KT, 128], F8, kind="ExternalInput")
    Woc = nc.dram_tensor("Woc", [OT, 128, 2, KT, 128], F8, kind="ExternalInput")
    Wkey8 = nc.dram_tensor("Wkey8", [UPT, 128, KT, 128], F8, kind="ExternalInput")
    Wvc = nc.dram_tensor("Wvc", [OT, 128, UPT + KT, 128], F8,
                         kind="ExternalInput")
    mixa = nc.dram_tensor("mixa", [128, 3, KT], F32, kind="ExternalInput")
    out = nc.dram_tensor("out", [OT, 128, 1024], F32, kind="ExternalOutput")
    x1f = nc.dram_tensor("x1f", [OT, 128, TCORE], F16, kind="Internal")

    with tile.TileContext(nc) as tc, contextlib.ExitStack() as g:
        cpool = g.enter_context(tc.tile_pool(name="consts", bufs=1))
        psg = contextlib.ExitStack()
        st = psg.enter_context(tc.tile_pool(name="st", bufs=1, space="PSUM"))
        mm = psg.enter_context(tc.tile_pool(name="mm", bufs=6, space="PSUM"))
        rows = g.enter_context(tc.tile_pool(name="rows", bufs=2))
        rsc = g.enter_context(tc.tile_pool(name="rsc", bufs=1))
        bcs = g.enter_context(tc.tile_pool(name="bcs", bufs=2))
        sqp = g.enter_context(tc.tile_pool(name="sqp", bufs=1))
        ones_r = cpool.tile([1, 128], F16)
        nc.vector.memset(ones_r[:], 1.0)
        ones8 = cpool.tile([128, 2, 128], F8)
        nc.vector.memset(ones8[:], 1.0)
        mixt = cpool.tile([128, 3, KT], F32)
        nc.sync.dma_start(mixt[:], mixa[:])
        mv = mixt[:, 0]
        mr = mixt[:, 1]
        mk = mixt[:, 2]

        def stats_rows(src8, n, sq_pre=None, split_sq=False):
            s1 = st.tile([128, 512], F32, tag="s1")
            s2 = st.tile([128, 512], F32, tag="s2")
            if sq_pre is None:
                sq = sqp.tile([128, KT, 342], F8, tag="sq")
                for ki in range(KT):
                    # split_sq alternates the squares between Act and DVE:
                    # at kernel start the serial 16-square Act chain gates
                    # LN1 stats -> bcast -> applies -> everything
                    if split_sq and ki % 2 == 1:
                        nc.vector.tensor_mul(sq[:, ki, :n], src8[:, ki, :],
                                             src8[:, ki, :])
                    else:
                        nc.scalar.square(sq[:, ki, :n], src8[:, ki, :])
            else:
                sq = sq_pre
            for ki in range(0, KT, 2):
                nc.tensor.matmul(s1[:, :n], ones8[:], src8[:, ki : ki + 2, :],
                                 start=(ki == 0), stop=(ki == KT - 2),
                                 perf_mode=DR)
            for ki in range(0, KT, 2):
                nc.tensor.matmul(s2[:, :n], ones8[:], sq[:, ki : ki + 2, :n],
                                 start=(ki == 0), stop=(ki == KT - 2),
                                 perf_mode=DR)
            m = rsc.tile([1, 512], F32, tag="m")
            nc.vector.tensor_scalar_mul(m[:, :n], s1[0:1, :n], INV_H)
            var = rsc.tile([1, 512], F32, tag="var")
            nc.vector.tensor_scalar_mul(var[:, :n], s2[0:1, :n], INV_H)
            msq = rsc.tile([1, 512], F32, tag="msd")
            nc.vector.tensor_mul(msq[:, :n], m[:, :n], m[:, :n])
            nc.vector.tensor_sub(var[:, :n], var[:, :n], msq[:, :n])
            nc.vector.tensor_scalar_add(var[:, :n], var[:, :n], EPS)
            sd = rsc.tile([1, 512], F32, tag="msd")
            nc.scalar.sqrt(sd[:, :n], var[:, :n])
            a_rf = rsc.tile([1, 512], F32, tag="var")
            nc.vector.reciprocal(a_rf[:, :n], sd[:, :n])
            a_rc = rows.tile([1, 512], F16, tag="arow")
            nc.vector.tensor_copy(a_rc[:, :n], a_rf[:, :n])
            c_rc = rows.tile([1, 512], F16, tag="crow")
            nc.vector.scalar_tensor_tensor(
                c_rc[:, :n], m[:, :n], -1.0, a_rf[:, :n],
                op0=OP.mult, op1=OP.mult)
            return a_rc, c_rc

        def bcast2(a_rc, c_rc, n, prev, hn):
            off = hn - n
            abp = st.tile([128, 512], F32, tag="s1")
            cbp = st.tile([128, 512], F32, tag="s2")
            if off:
                pa_rc, pc_rc, pn = prev
                nc.tensor.matmul(abp[:, 0:1], ones_r[:], pa_rc[:, pn - 1 : pn],
                                 start=True, stop=True, skip_group_check=True)
                nc.tensor.matmul(cbp[:, 0:1], ones_r[:], pc_rc[:, pn - 1 : pn],
                                 start=True, stop=True, skip_group_check=True)
            nc.tensor.matmul(abp[:, off : off + n], ones_r[:], a_rc[:, :n],
                             start=True, stop=True, skip_group_check=True)
            nc.tensor.matmul(cbp[:, off : off + n], ones_r[:], c_rc[:, :n],
                             start=True, stop=True, skip_group_check=True)
            ab = bcs.tile([128, 512], F16, tag="ab")
            nc.scalar.copy(ab[:, :hn], abp[:, :hn])
            cb = bcs.tile([128, 512], F16, tag="cb")
            nc.scalar.copy(cb[:, :hn], cbp[:, :hn])
            return ab, cb

        with tc.tile_pool(name="cmp", bufs=1) as cmp_:
            cmt = cmp_.tile([128, KT, 1024], F8)

            # ohi/olo/vhi/vlo are Act/Pool-written only: keeping them first
            # gives the FFN sil tile a 64KB window free of DVE/DMA-written
            # space (cross-engine waits snapshot full engine counts, so any
            # DVE/DMA overlap would stall the silus behind the LN2 tail).
            with tc.tile_pool(name="ohip", bufs=1) as ohip, \
                 tc.tile_pool(name="olop", bufs=1) as olop, \
                 tc.tile_pool(name="vhip", bufs=1) as vhip, \
                 tc.tile_pool(name="vlop", bufs=1) as vlop, \
                 tc.tile_pool(name="xtlp", bufs=1) as xtlp:
                ohi = [ohip.tile([128, KT, pb - pa], F8, tag=f"ohi{ci}",
                                 name=f"ohi{ci}")
                       for ci, (pa, pb) in enumerate(CH)]
                olo = [olop.tile([128, KT, pb - pa], F8, tag=f"olo{ci}",
                                 name=f"olo{ci}")
                       for ci, (pa, pb) in enumerate(CH)]
                vhi = [vhip.tile([128, KT, pb - pa], F8, tag=f"vhi{ci}",
                                 name=f"vhi{ci}")
                       for ci, (pa, pb) in enumerate(CH)]
                vlo = [vlop.tile([128, KT, pb - pa], F8, tag=f"vlo{ci}",
                                 name=f"vlo{ci}")
                       for ci, (pa, pb) in enumerate(CH)]
                xtl = []
                for ci, (sa, sb) in enumerate(ST1):
                    ha = max(sa - 1, 0)
                    xtl.append(xtlp.tile([128, KT, sb - ha], F16,
                                         tag=f"xtl{ci}", name=f"xtl{ci}"))

                with tc.tile_pool(name="x8p", bufs=1) as x8p, \
                     tc.tile_pool(name="rinp", bufs=1) as rinp, \
                     tc.tile_pool(name="p1sc", bufs=2) as p1sc, \
                     tc.tile_pool(name="hp", bufs=2) as hp, \
                     tc.tile_pool(name="wvrp", bufs=5) as wvrp, \
                     tc.tile_pool(name="sgp", bufs=3) as sgp, \
                     tc.tile_pool(name="vsbp", bufs=3) as vsbp, \
                     tc.tile_pool(name="oisc", bufs=2) as oisc:
                    rin = [rinp.tile([128, KT, pb - pa], F8, tag=f"rin{ci}",
                                     name=f"rin{ci}")
                           for ci, (pa, pb) in enumerate(CH)]
                    x8l = [x8p.tile([128, KT, 342], F8, tag=f"x8l{ci}",
                                    name=f"x8l{ci}") for ci in range(3)]

                    def load_x8(ci):
                        nc.sync.dma_start(x8l[ci][:], xT8[ci])

                    def load_xtl(ci):
                        sa, sb = ST1[ci]
                        ha = max(sa - 1, 0)
                        nc.sync.dma_start(xtl[ci][:], xT[:, :, ha:sb])

                    load_x8(0)
                    load_xtl(0)
                    ln1 = {}

                    def ln1_rowsbc(ci, prev):
                        sa, sb = ST1[ci]
                        ha = max(sa - 1, 0)
                        hn = sb - ha
                        n = sb - sa
                        a_rc, c_rc = stats_rows(x8l[ci][:, :, :n], n,
                                                split_sq=(ci == 0))
                        ln1[ci] = (bcast2(a_rc, c_rc, n, prev, hn), ha, hn)
                        return (a_rc, c_rc, n)

                    def ln1_apply_pair(ci, kp, pool_mul=False):
                        """Batched LN1 apply + mix for ki = 2kp, 2kp+1.

                        (pool_mul is unused: routing the rin STTs or the
                        mul to GpSimd fails neuronxcc's engine check for
                        TensorScalarPtr / was Pool-pace-limited.)
                        """
                        (ab, cb), ha, hn = ln1[ci]
                        nmix = hn - 1
                        k0 = 2 * kp
                        ab_b = ab[:, :hn].unsqueeze(1).broadcast_to(
                            [128, 2, hn])
                        cb_b = cb[:, :hn].unsqueeze(1).broadcast_to(
                            [128, 2, hn])
                        tt = p1sc.tile([128, 2, hn], F16, tag="tt")
                        nc.vector.tensor_mul(tt[:], xtl[ci][:, k0 : k0 + 2, :],
                                             ab_b)
                        h = hp.tile([128, 2, hn], F16, tag="h")
                        nc.vector.tensor_add(h[:], tt[:], cb_b)
                        d = p1sc.tile([128, 2, hn], F16, tag="d")
                        hf = h[:].rearrange("p a b -> p (a b)")
                        df = d[:].rearrange("p a b -> p (a b)")
                        nc.vector.tensor_sub(df[:, : 2 * hn - 1],
                                             hf[:, 1 : 2 * hn],
                                             hf[:, : 2 * hn - 1])
                        v16 = p1sc.tile([128, 2, nmix], F16, tag="v16")
                        for j in range(2):
                            ki = k0 + j
                            nc.vector.scalar_tensor_tensor(
                                v16[:, j, :], d[:, j, :nmix],
                                mv[:, ki : ki + 1], h[:, j, :nmix],
                                op0=OP.mult, op1=OP.add)
                            nc.vector.scalar_tensor_tensor(
                                rin[ci][:, ki, :], d[:, j, :nmix],
                                mr[:, ki : ki + 1], h[:, j, :nmix],
                                op0=OP.mult, op1=OP.add)
                        nc.scalar.copy(vhi[ci][:, k0 : k0 + 2, :], v16[:])
                        nc.gpsimd.tensor_sub(vlo[ci][:, k0 : k0 + 2, :],
                                             v16[:], vhi[ci][:, k0 : k0 + 2, :])

                    def mm_vr(ci, extra=None):
                        pa, pb = CH[ci]
                        n = pb - pa
                        oi16 = {}
                        for oi in range(OT):
                            wt = wvrp.tile([128, 2, KT, 128], F8, tag="wvr")
                            nc.sync.dma_start(wt[:], Wvr[oi])
                            vps = mm.tile([128, 512], F32, tag="acc")
                            for ki in range(0, KT, 2):
                                nc.tensor.matmul(
                                    vps[:, :n], wt[:, 0, ki : ki + 2, :],
                                    vhi[ci][:, ki : ki + 2, :],
                                    start=(ki == 0), stop=False, perf_mode=DR)
                            for ki in range(0, KT, 2):
                                nc.tensor.matmul(
                                    vps[:, :n], wt[:, 0, ki : ki + 2, :],
                                    vlo[ci][:, ki : ki + 2, :],
                                    start=False, stop=(ki == KT - 2),
                                    perf_mode=DR)
                            rps = mm.tile([128, 512], F32, tag="acc")
                            for ki in range(0, KT, 2):
                                nc.tensor.matmul(
                                    rps[:, :n], wt[:, 1, ki : ki + 2, :],
                                    rin[ci][:, ki : ki + 2, :],
                                    start=(ki == 0), stop=(ki == KT - 2),
                                    perf_mode=DR)
                            # interleaved work issued AFTER this oi's chains:
                            # count-based semaphores make a chain wait on any
                            # DVE work issued before it, so issuing the next
                            # chunk's apply first would serialize PE behind it
                            if extra is not None:
                                extra(oi)
                            sg = sgp.tile([128, 512], F16, tag="sg")
                            nc.scalar.activation(sg[:, :n], rps[:, :n],
                                                 AF.Sigmoid, scale=ISW)
                            vsb = vsbp.tile([128, 512], F16, tag="vsb")
                            nc.scalar.activation(vsb[:, :n], vps[:, :n],
                                                 AF.Copy, scale=ISW)
                            if oi % 2 == 0:
                                oi16[oi // 2] = oisc.tile(
                                    [128, 2, n], F16, tag="oi16",
                                    name=f"oi16_{ci}_{oi}")
                            cur = oi16[oi // 2]
                            nc.vector.tensor_mul(cur[:, oi % 2, :],
                                                 sg[:, :n], vsb[:, :n])
                            if oi % 2 == 1:
                                o0 = oi - 1
                                nc.scalar.copy(ohi[ci][:, o0 : o0 + 2, :],
                                               cur[:])
                                nc.gpsimd.tensor_sub(
                                    olo[ci][:, o0 : o0 + 2, :], cur[:],
                                    ohi[ci][:, o0 : o0 + 2, :])

                    pr = ln1_rowsbc(0, None)
                    for kp in range(KT // 2):
                        ln1_apply_pair(0, kp, pool_mul=True)
                    load_x8(1)
                    pr1 = ln1_rowsbc(1, pr)
                    load_xtl(1)

                    def extra_c1(oi):
                        if oi == 8:
                            load_x8(2)
                        if oi == 12:
                            load_xtl(2)
                        if oi % 2 == 1:
                            ln1_apply_pair(1, oi // 2)

                    mm_vr(0, extra=extra_c1)
                    ln1_rowsbc(2, pr1)
                    mm_vr(1, extra=lambda oi: (
                        ln1_apply_pair(2, oi // 2) if oi % 2 == 1 else None))
                    mm_vr(2)

                # ---------- o-proj + residual + LN2 ----------
                # pool order fixes SBUF address reuse for the FFN pools
                # stacked after these pop: the d_apply scratch (dsc/h2p/cyp,
                # DVE-written until the chunk-2 LN2 tail ends) must land
                # under wvcp (first written in the val phase), never under
                # silup — else the first silu inherits an ~18us wait.
                # address order (low→high): x2b8 (Act-written) and wop (DMA)
                # land under the FFN's wkp/wvcp claims; x2b and the d_apply
                # scratch sit above everything the FFN pools reach, so no FFN
                # write inherits a wait on the LN2 tail or the spills.
                ost = contextlib.ExitStack()
                x2b8p = ost.enter_context(tc.tile_pool(name="x2b8", bufs=2))
                wop = ost.enter_context(tc.tile_pool(name="wop", bufs=4))
                x2bp = ost.enter_context(tc.tile_pool(name="x2b", bufs=1))
                dsc = ost.enter_context(tc.tile_pool(name="dsc", bufs=3))
                h2p = ost.enter_context(tc.tile_pool(name="h2p", bufs=2))
                cyp = ost.enter_context(tc.tile_pool(name="cyp", bufs=1))
                x2b = {}
                x2b8 = {}
                ln2 = {}
                carry = {}
                for ci in range(3):
                    x2b[ci] = x2bp.tile([128, KT, 342], F16,
                                        tag=f"x2b{ci}", name=f"x2b{ci}")
                    carry[ci] = cyp.tile([128, KT], F16, tag=f"cy{ci}",
                                         name=f"cy{ci}")

                def c_step(ci, oi):
                    pa, pb = CH[ci]
                    sa, sb = ST1[ci]
                    ha = max(sa - 1, 0)
                    n = pb - pa
                    if oi == 0:
                        x2b8[ci] = x2b8p.tile([128, KT, 342], F8,
                                              tag="x2b8", name=f"x2b8{ci}")
                    wt = wop.tile([128, 2, KT, 128], F8, tag="wo")
                    nc.sync.dma_start(wt[:], Woc[oi])
                    ops_ = mm.tile([128, 512], F32, tag="acc")
                    for ki in range(0, KT, 2):
                        nc.tensor.matmul(
                            ops_[:, :n], wt[:, 0, ki : ki + 2, :],
                            ohi[ci][:, ki : ki + 2, :],
                            start=(ki == 0), stop=False, perf_mode=DR)
                    for ki in range(0, KT, 2):
                        nc.tensor.matmul(
                            ops_[:, :n], wt[:, 1, ki : ki + 2, :],
                            ohi[ci][:, ki : ki + 2, :],
                            start=False, stop=False, perf_mode=DR)
                    for ki in range(0, KT, 2):
                        nc.tensor.matmul(
                            ops_[:, :n], wt[:, 0, ki : ki + 2, :],
                            olo[ci][:, ki : ki + 2, :],
                            start=False, stop=(ki == KT - 2), perf_mode=DR)
                    nc.vector.scalar_tensor_tensor(
                        x2b[ci][:, oi, :n], ops_[:, :n], ISW,
                        xtl[ci][:, oi, pa - ha : pb - ha],
                        op0=OP.mult, op1=OP.add)
                    nc.scalar.copy(x2b8[ci][:, oi, :n], x2b[ci][:, oi, :n])

                def d_rowsbc(ci, sq_pre=None):
                    pa, pb = CH[ci]
                    n = pb - pa
                    a_rc, c_rc = stats_rows(x2b8[ci][:, :, :n], n,
                                            sq_pre=sq_pre)
                    ln2[ci] = bcast2(a_rc, c_rc, n, None, n)

                def d_apply_pair(ci, kp):
                    """Batched LN2 apply + mix for ki = 2kp, 2kp+1.

                    Chunks 0/1 put the x*ab multiply on GpSimd (idle during
                    the c-phases).  Chunk 2 stays entirely on DVE: its tail
                    runs concurrently with the first FFN silus, and any Pool
                    work issued there would stall them via the sil tile's
                    count-based Pool anti-dependency (sil sits over the
                    Pool-written olo/vlo ranges).
                    """
                    pa, pb = CH[ci]
                    n = pb - pa
                    k0 = 2 * kp
                    ab, cb = ln2[ci]
                    ab_b = ab[:, :n].unsqueeze(1).broadcast_to([128, 2, n])
                    cb_b = cb[:, :n].unsqueeze(1).broadcast_to([128, 2, n])
                    tt = dsc.tile([128, 2, n], F16, tag="tt2")
                    nc.vector.tensor_mul(tt[:], x2b[ci][:, k0 : k0 + 2, :n],
                                         ab_b)
                    h2 = h2p.tile([128, 2, n], F16, tag="h2")
                    nc.vector.tensor_add(h2[:], tt[:], cb_b)
                    if ci > 0:
                        pcy = carry[ci - 1]
                        db = dsc.tile([128, 2], F16, tag="db")
                        nc.vector.tensor_sub(
                            db[:], h2[:, :, 0], pcy[:, k0 : k0 + 2])
                        gidx = pa - 2
                        for j in range(2):
                            ki = k0 + j
                            nc.vector.scalar_tensor_tensor(
                                cmt[:, ki, gidx : gidx + 1], db[:, j : j + 1],
                                mk[:, ki : ki + 1], pcy[:, ki : ki + 1],
                                op0=OP.mult, op1=OP.add)
                    d2 = dsc.tile([128, 2, n], F16, tag="d2")
                    h2f = h2[:].rearrange("p a b -> p (a b)")
                    d2f = d2[:].rearrange("p a b -> p (a b)")
                    nc.vector.tensor_sub(d2f[:, : 2 * n - 1],
                                         h2f[:, 1 : 2 * n],
                                         h2f[:, : 2 * n - 1])
                    glo, ghi = pa - 1, pb - 2
                    for j in range(2):
                        ki = k0 + j
                        nc.vector.scalar_tensor_tensor(
                            cmt[:, ki, glo:ghi], d2[:, j, : ghi - glo],
                            mk[:, ki : ki + 1], h2[:, j, : ghi - glo],
                            op0=OP.mult, op1=OP.add)
                    nc.vector.tensor_copy(carry[ci][:, k0 : k0 + 2],
                                          h2[:, :, n - 1])

                def spill_x1f(ci):
                    pa, pb = CH[ci]
                    n = pb - pa
                    nc.sync.dma_start(
                        x1f[:, :, pa:pb].transpose([1, 0, 2]),
                        x2b[ci][:, :, :n])

                for oi in range(OT):
                    c_step(0, oi)
                spill_x1f(0)
                d_rowsbc(0)
                for oi in range(OT):
                    c_step(1, oi)
                    if oi % 2 == 1:
                        d_apply_pair(0, oi // 2)
                spill_x1f(1)
                d_rowsbc(1)
                # chunk-2 stats squares computed per-oi inside the loop so
                # d_rowsbc(2) doesn't serialize a 16-square Act chain at the
                # FFN boundary
                sq2 = sqp.tile([128, KT, 342], F8, tag="sq")
                n2 = CH[2][1] - CH[2][0]
                for oi in range(OT):
                    c_step(2, oi)
                    nc.scalar.square(sq2[:, oi, :n2], x2b8[2][:, oi, :n2])
                    if oi % 2 == 1:
                        d_apply_pair(1, oi // 2)
                spill_x1f(2)
                d_rowsbc(2, sq_pre=sq2)
                for kp in range(KT // 2):
                    d_apply_pair(2, kp)
                ost.close()

            # ---------- FFN single-pass ----------
            psg.close()
            # pool creation order fixes SBUF address reuse: wkp+wvcp (whose
            # first tiles are written late or wait harmlessly) land on the
            # o-proj scratch ranges still being read by the LN2 tail; sil
            # lands above them on long-dead TM space.  Otherwise the first
            # silu inherits a wait on the entire chunk-2 d_apply chain.
            with tc.tile_pool(name="mme", bufs=8, space="PSUM") as mme, \
                 tc.tile_pool(name="silup", bufs=1) as silup, \
                 tc.tile_pool(name="wkp", bufs=KG_HEAD) as wkp, \
                 tc.tile_pool(name="wvcp", bufs=3) as wvcp, \
                 tc.tile_pool(name="fsc", bufs=2) as fsc, \
                 tc.tile_pool(name="prp", bufs=2) as prp:
                sil = silup.tile([128, UPT, 1024], F8)
                wk_head = {}

                def load_wk(gi):
                    wk = wkp.tile([128, KG, KT, 128], F8, tag="wkg",
                                  name=f"wkg{gi}")
                    nc.sync.dma_start(wk[:], Wkey8[gi * KG : (gi + 1) * KG]
                                      .transpose([1, 0, 2, 3]))
                    return wk

                def key_chain(wk_g, gi, ui, cs, cw):
                    kps = mme.tile([128, 512], F32, tag="acc")
                    for ki in range(0, KT, 2):
                        nc.tensor.matmul(
                            kps[:, :cw], wk_g[:, ui - gi * KG, ki : ki + 2, :],
                            cmt[:, ki : ki + 2, cs : cs + cw],
                            start=(ki == 0), stop=(ki == KT - 2), perf_mode=DR)
                    nc.scalar.activation(sil[:, ui, cs : cs + cw],
                                         kps[:, :cw], AF.Silu, scale=ISW)

                # Pass A: columns 0..682 need only LN2 chunks 0/1 (done before
                # the FFN starts), so all of it runs while the chunk-2 LN2
                # apply tail finishes on DVE.  Pass B (columns 683..1023)
                # starts from the still-resident last groups (no DMA) and
                # re-streams the earlier groups.  Weight loads stay 1 group
                # ahead: cross-engine waits count ALL DMAs issued so far, so
                # a front-loaded burst would stall the silus.
                pending = load_wk(0)
                for gi in range(NKG):
                    wk_g = pending
                    wk_head[gi] = wk_g
                    if gi + 1 < NKG:
                        pending = load_wk(gi + 1)
                    for ui in range(gi * KG, (gi + 1) * KG):
                        key_chain(wk_g, gi, ui, 0, 512)
                    for ui in range(gi * KG, (gi + 1) * KG):
                        key_chain(wk_g, gi, ui, 512, 170)
                # last 5 groups still resident in wkp's bufs; consume them in
                # slot-rotation order so each reload can start as soon as its
                # slot's reader finishes
                for gi in range(NKG - 5, NKG):
                    for ui in range(gi * KG, (gi + 1) * KG):
                        key_chain(wk_head[gi], gi, ui, 682, 342)
                pending = load_wk(0)
                for gi in range(NKG - 5):
                    wk_g = pending
                    if gi + 1 < NKG - 5:
                        pending = load_wk(gi + 1)
                    for ui in range(gi * KG, (gi + 1) * KG):
                        key_chain(wk_g, gi, ui, 682, 342)

                for oi in range(OT):
                    wt = wvcp.tile([128, UPT + KT, 128], F8, tag="wvc")
                    nc.sync.dma_start(wt[:], Wvc[oi])
                    x1t = fsc.tile([128, 1024], F16, tag="x1r")
                    nc.sync.dma_start(x1t[:], x1f[oi, :, 2 : 2 + 1024])
                    prod = prp.tile([128, 1024], F32, tag="prod")
                    for hf in range(2):
                        cs = hf * 512
                        kvps = mme.tile([128, 512], F32, tag="acc")
                        for ki in range(0, UPT, 2):
                            nc.tensor.matmul(
                                kvps[:], wt[:, ki : ki + 2, :],
                                sil[:, ki : ki + 2, cs : cs + 512],
                                start=(ki == 0), stop=(ki == UPT - 2),
                                perf_mode=DR)
                        rrps = mme.tile([128, 512], F32, tag="acc")
                        for ki in range(0, KT, 2):
                            nc.tensor.matmul(
                                rrps[:], wt[:, UPT + ki : UPT + ki + 2, :],
                                cmt[:, ki : ki + 2, cs : cs + 512],
                                start=(ki == 0), stop=(ki == KT - 2),
                                perf_mode=DR)
                        # the very last epilogue runs in quarters so the
                        # end-of-kernel drain waits on a 256-col chain, not
                        # a 512-col one
                        nq = 2 if (oi == OT - 1 and hf == 1) else 1
                        qw = 512 // nq
                        sr = fsc.tile([128, 512], F16, tag="sr")
                        for q in range(nq):
                            qs = cs + q * qw
                            nc.scalar.activation(sr[:, q * qw : q * qw + qw],
                                                 rrps[:, q * qw : q * qw + qw],
                                                 AF.Sigmoid, scale=ISW)
                            nc.vector.scalar_tensor_tensor(
                                prod[:, qs : qs + qw],
                                kvps[:, q * qw : q * qw + qw], ISWV,
                                sr[:, q * qw : q * qw + qw],
                                op0=OP.mult, op1=OP.mult)
                            nc.vector.tensor_add(prod[:, qs : qs + qw],
                                                 prod[:, qs : qs + qw],
                                                 x1t[:, qs : qs + qw])
                            # per-part out spill overlaps the following work,
                            # shortening the end-of-kernel drain
                            nc.sync.dma_start(out[oi][:, qs : qs + qw],
                                              prod[:, qs : qs + qw])
    nc.compile()
    return nc


def get_nc():
    if "nc" not in _BUILD_CACHE:
        _BUILD_CACHE["nc"] = build()
    return _BUILD_CACHE["nc"]


def make_in_maps(inputs):
    x = np.asarray(inputs["x"], dtype=np.float32)
    Wv8 = _p8(inputs["Wv"], SW)
    Wo8, Wolo = _p8_hilo(inputs["Wo"], SW)
    Wr8 = _p8(inputs["Wr"], SW)
    # combined panels: [OT, 128, parts, KT, 128]
    Wvr = np.ascontiguousarray(np.stack([Wv8, Wr8], axis=2))
    Woc = np.ascontiguousarray(np.stack([Wo8, Wolo], axis=2))
    Wval8 = _p8(inputs["Wval"], SWV)         # [OT, 128, UPT, 128]
    Wcr8 = _p8(inputs["Wcr"], SW)            # [OT, 128, KT, 128]
    Wvc = np.ascontiguousarray(np.concatenate([Wval8, Wcr8], axis=2))
    shared = {
        "Wvr": Wvr, "Woc": Woc,
        "Wkey8": np.ascontiguousarray(_p8(inputs["Wkey"], SW)),
        "Wvc": Wvc,
        "mixa": np.ascontiguousarray(np.stack(
            [_mix128(inputs["tm_mv"]), _mix128(inputs["tm_mr"]),
             _mix128(inputs["cm_mk"])], axis=1)),
    }
    in_maps = []
    for c in range(8):
        b, half = divmod(c, 2)
        s = half * 1024
        xs = np.zeros((TCORE, H), np.float32)
        lo = max(s - 2, 0)
        xs[2 - (s - lo):, :] = x[b, lo : s + 1024, :]
        xs16 = xs.T.astype(np.float16)                     # [H, TCORE]
        xp = xs16.reshape(KT, 128, TCORE).transpose(1, 0, 2)
        m = dict(shared)
        m["xT"] = np.ascontiguousarray(xp)
        xp8 = xp.astype(E4)
        m["xT8"] = np.ascontiguousarray(
            np.stack([xp8[:, :, 0:342], xp8[:, :, 342:684],
                      xp8[:, :, 684:1026]]))
        in_maps.append(m)
    return in_maps


def run(inputs, **kw):
    from concourse.bass_utils import run_bass_kernel_spmd

    in_maps = make_in_maps(inputs)
    nc = get_nc()
    res = run_bass_kernel_spmd(nc, in_maps, core_ids=list(range(8)), **kw)
    outa = np.empty((B, T, H), np.float32)
    for c in range(8):
        b, half = divmod(c, 2)
        o = res.results[c]["out"].reshape(H, 1024)
        outa[b, half * 1024 : (half + 1) * 1024, :] = o.T
    return outa, res


def kernel(**inputs):
    return run(inputs)[0]
